# revision 35
# baseline (speedup 1.0000x reference)
"""Trainium2 Bass kernel for nn_HOPE_7275674599449.

Decay-masked fast-weight attention + 4-layer MLP stack + LM head,
data-parallel over 8 NeuronCores (512 tokens each, 128-token halo for
the decay-banded attention; decay^128 underflows fp32 so the banding
is numerically exact).

Per-core program (feature-major activations [d_partitions, tokens]):
  - q/k/v/o projections + scores + attn in f32r (fp22 on PE, 1 cyc/row)
  - MLP + LM head matmuls in fp8e4 DoubleRow (0.5 cyc/row, 2 k-tiles
    per instruction) with an error-compensated split:
        W@h = W8@h8 + (dW8@h8 + W8@dh8),  W8 = fp8(128*W), dW8 = fp8(128*W - W8)
    The two correction products share one DoubleRow instruction per
    k-tile, so the whole thing costs 12 slot-pairs per 8 k-tiles =
    0.75x bf16 while landing ~bf16 accuracy (measured rel ~3e-3).
  - LayerNorms: partition-dim reductions via ones-matmul on PE (f32r),
    per-token stats broadcast via ones-matmul, elementwise on DVE/ACT.
  - Token-half (A/B) software pipeline: the o-proj / MLP matmuls and
    each LayerNorm are split into 256-token halves and emitted in a
    shifted order, so the LN + fp8-conversion chain of one half runs
    on DVE/ACT/Pool while the PE crunches the other half.
"""

import sys

sys.path.insert(0, "/opt/trn_rl_repo")

from contextlib import ExitStack

import ml_dtypes
import numpy as np

import concourse.bass as bass
import concourse.tile as tile
from concourse import bacc, mybir
from concourse.bass_utils import run_bass_kernel_spmd

P = 128
B, S, D, L, V = 2, 2048, 1024, 4, 32000
ED = 4 * D              # MLP hidden
CH = 512                # tokens per core
TT = CH // 2            # token half
HALO = 128
WIN = HALO + CH         # 640
KD = D // P             # 8
KU = ED // P            # 32
MW = WIN // P           # 5 window token chunks
NV = 500                # head free-dim chunk
NVC = V // NV           # 64
NCORES = 8
EPS = 1e-5
SW = 128.0              # fp8 weight pre-scale (power of 2, exact)

f32 = mybir.dt.float32
f32r = mybir.dt.float32r
bf16 = mybir.dt.bfloat16
fp8 = mybir.dt.float8e4
DRM = mybir.MatmulPerfMode.DoubleRow
E4NP = ml_dtypes.float8_e4m3   # TRN e4m3 (max 240)

# smalls stacking indices (rows of the [18, D] f32 "smalls" tensor)
I_BQ, I_BK, I_BV, I_BO, I_GF, I_BF = 0, 1, 2, 3, 4, 5
def I_B2(l): return 6 + 3 * l
def I_GC(l): return 7 + 3 * l
def I_BE(l): return 8 + 3 * l
NS = 6 + 3 * L

TRACE = False          # set by test.py for profiled runs
_CACHE = {}


def _halves_seq(n, shift):
    """Emission order (half, m): A leads B by `shift` m-chunks."""
    seq = [(0, m) for m in range(min(shift, n))]
    for m in range(shift, n):
        seq.append((1, m - shift))
        seq.append((0, m))
    for m in range(max(0, n - shift), n):
        seq.append((1, m))
    return seq


def _build_program(flags):
    """Build the per-core Bass/Tile program. flags: dict of use_* booleans."""
    nc = bacc.Bacc("TRN2", target_bir_lowering=False, debug=False,
                   num_devices=NCORES)

    xw = nc.dram_tensor("xw", [D, WIN], f32r, kind="ExternalInput").ap()
    maskT = nc.dram_tensor("maskT", [WIN, CH], f32, kind="ExternalInput").ap()
    wq = nc.dram_tensor("wq", [KD, D, P], f32r, kind="ExternalInput").ap()
    wk = nc.dram_tensor("wk", [KD, D, P], f32r, kind="ExternalInput").ap()
    wo = nc.dram_tensor("wo", [KD, D, P], f32r, kind="ExternalInput").ap()
    wv = nc.dram_tensor("wv", [D, D], f32r, kind="ExternalInput").ap()
    onesc = nc.dram_tensor("onesc", [P, 1], f32r, kind="ExternalInput").ap()
    onesr = nc.dram_tensor("onesr", [1, P], f32r, kind="ExternalInput").ap()
    # fp8 weight pairs: b=0 -> d (fp8 of scaled residual), b=1 -> hi fp8
    w1c = nc.dram_tensor("w1c", [L, KU // 4, P, 4, 2, KD, P], fp8,
                         kind="ExternalInput").ap()
    w2c = nc.dram_tensor("w2c", [L, KD, P, 2, KU, P], fp8, kind="ExternalInput").ap()
    whc = nc.dram_tensor("whc", [NVC, P, 2, KD, NV], fp8, kind="ExternalInput").ap()
    smalls = nc.dram_tensor("smalls", [P, NS, KD], f32, kind="ExternalInput").ap()
    b1v = nc.dram_tensor("b1v", [P, L, KU], f32, kind="ExternalInput").ap()
    bhv = None
    if flags["use_bh"]:
        bhv = nc.dram_tensor("bhv", [V], f32, kind="ExternalInput").ap()
    bvv = None
    if flags["use_bv"]:
        bvv = nc.dram_tensor("bvv", [D], f32, kind="ExternalInput").ap()
    out = nc.dram_tensor("out", [CH, V], bf16, kind="ExternalOutput").ap()

    # f32r tiles: PE reads fp32 bits, truncates to fp22, 1 cyc/row (vs 4
    # for fp32) when the moving free dim is >=256. The BIR verifier wants
    # every producer of an f32r-matmul operand typed f32r, so the tiles are
    # declared f32r and elementwise engines read them via .bitcast(f32).
    def c(ap): return ap.bitcast(f32)

    def tsl(t2):
        return slice(TT * t2, TT * (t2 + 1))

    with tile.TileContext(nc) as tc, ExitStack() as ctx:
        persist = ctx.enter_context(tc.tile_pool(name="persist", bufs=1))
        sqp = ctx.enter_context(tc.tile_pool(name="sqp", bufs=4))
        lnt = ctx.enter_context(tc.tile_pool(name="lnt", bufs=3))
        psum_mm = ctx.enter_context(
            tc.tile_pool(name="psum_mm", bufs=5, space="PSUM"))
        psum_s = ctx.enter_context(
            tc.tile_pool(name="psum_s", bufs=2, space="PSUM"))
        psum_bc = ctx.enter_context(
            tc.tile_pool(name="psum_bc", bufs=1, space="PSUM"))

        h = persist.tile([P, KD, CH], f32r)
        # hc: fp8 pair of h. [:,0,k,:] = h8, [:,1,k,:] = dh8 = fp8(h - h8)
        hc = persist.tile([P, 2, KD, CH], fp8)
        sm = persist.tile([P, NS, KD], f32)
        b1s = persist.tile([P, L, KU], f32)
        ones_col = persist.tile([P, 1], f32r)
        ones_row = persist.tile([1, P], f32r)
        eps_t = persist.tile([1, 1], f32)
        nc.vector.memset(eps_t, EPS)
        zero_b = persist.tile([P, 1], f32)
        nc.vector.memset(zero_b, 0.0)
        inv_sw = persist.tile([P, 1], f32)
        nc.vector.memset(inv_sw, 1.0 / SW)


        def bias_ap(idx, k):
            return sm[:, idx, k:k + 1]

        def fp8_pair(dst8, dstd, src):
            """dst8 = fp8(src); dstd = fp8(src - dst8). DVE does the sub."""
            with nc.allow_low_precision(reason="fp8 pair for DoubleRow"):
                nc.gpsimd.tensor_copy(dst8, src)
                nc.vector.tensor_sub(dstd, src, dst8)

        def fp8_pair_pool(dst8, dstd, src):
            """Same, entirely on Pool: keeps the LN-critical DVE free and
            avoids a cross-engine sem hop between copy and sub."""
            with nc.allow_low_precision(reason="fp8 pair for DoubleRow"):
                nc.gpsimd.tensor_copy(dst8, src)
                nc.gpsimd.tensor_sub(dstd, src, dst8)

        def layernorm_half(g_idx, b_idx, t2, apply_gb):
            """h[:, :, half] = LN(h)*g + b over d; refresh hc half."""
            sl = tsl(t2)
            ps_s = psum_s.tile([1, TT], f32, tag="lnsum")
            for k in range(KD):
                nc.tensor.matmul(ps_s, lhsT=ones_col, rhs=h[:, k, sl],
                                 start=(k == 0), stop=(k == KD - 1))
            ps_q = psum_s.tile([1, TT], f32, tag="lnsum")
            for k in range(KD):
                sq = sqp.tile([P, TT], f32r, tag="sq")
                nc.scalar.square(sq, c(h[:, k, sl]))
                nc.tensor.matmul(ps_q, lhsT=ones_col, rhs=sq,
                                 start=(k == 0), stop=(k == KD - 1))
            mean = lnt.tile([1, TT], f32r, tag="lnstat")
            nc.scalar.mul(mean, ps_s, 1.0 / D)
            ex2 = lnt.tile([1, TT], f32, tag="lnstat")
            nc.scalar.mul(ex2, ps_q, 1.0 / D)
            var = lnt.tile([1, TT], f32, tag="lnstat")
            nc.vector.tensor_mul(var, c(mean), c(mean))
            nc.vector.tensor_sub(var, ex2, var)
            std = lnt.tile([1, TT], f32, tag="lnstat")
            nc.scalar.activation(std, var,
                                 mybir.ActivationFunctionType.Sqrt, bias=eps_t)
            rstd = lnt.tile([1, TT], f32r, tag="lnstat")
            with nc.allow_low_precision(reason="f32r carries full fp32 bits"):
                nc.vector.reciprocal(rstd, std)
            bc2 = psum_bc.tile([P, 2, TT], f32, tag="bc")
            ps_mb, ps_rb = bc2[:, 0, :], bc2[:, 1, :]
            nc.tensor.matmul(ps_mb, lhsT=ones_row, rhs=mean,
                             start=True, stop=True)
            nc.tensor.matmul(ps_rb, lhsT=ones_row, rhs=rstd,
                             start=True, stop=True)
            for k in range(KD):
                t = lnt.tile([P, TT], f32, tag="lntmp")
                nc.vector.tensor_sub(t, c(h[:, k, sl]), ps_mb)
                if apply_gb:
                    nc.vector.tensor_mul(t, t, ps_rb)
                    nc.scalar.activation(h[:, k, sl], t,
                                         mybir.ActivationFunctionType.Identity,
                                         bias=bias_ap(b_idx, k),
                                         scale=bias_ap(g_idx, k))
                else:
                    # g==1, b==0: the normalized value IS h
                    with nc.allow_low_precision(reason="f32r=fp32 bits"):
                        nc.vector.tensor_mul(h[:, k, sl], t, ps_rb)
                fp8_pair_pool(hc[:, 0, k, sl], hc[:, 1, k, sl],
                              c(h[:, k, sl]))

        # ---------------- attention ----------------
        with tc.tile_pool(name="attn", bufs=1) as ap_, \
             tc.tile_pool(name="wqk", bufs=4) as wqk_pool, \
             tc.tile_pool(name="wvp", bufs=2) as wv_pool:
            xw_sb = ap_.tile([P, KD, WIN], f32r)
            wq_r = [None] * KD
            # first q-proj weight tile before everything else: it gates the
            # very first matmul
            wt0 = wqk_pool.tile([P, KD, P], f32r, tag="wqk")
            nc.sync.dma_start(out=wt0,
                              in_=wq[0].rearrange("(k p) c -> p k c", p=P))
            for k in range(KD):
                nc.sync.dma_start(
                    out=xw_sb[:, k, :],
                    in_=xw.rearrange("(k p) t -> p k t", p=P)[:, k, :])
            nc.sync.dma_start(out=ones_col, in_=onesc)
            nc.sync.dma_start(out=ones_row, in_=onesr)
            if flags["use_bv"]:
                bv_bc = ap_.tile([P, D], f32)
                src = bass.AP(tensor=bvv.tensor, offset=bvv.offset,
                              ap=[[0, P], bvv.ap[0]])
                nc.sync.dma_start(out=bv_bc, in_=src)

            # qT [d, q]
            qT = ap_.tile([P, KD, CH], f32r, tag="qslot")
            for m in range(KD):
                if m == 0:
                    wt = wt0
                else:
                    wt = wqk_pool.tile([P, KD, P], f32r, tag="wqk")
                    nc.sync.dma_start(out=wt,
                                      in_=wq[m].rearrange("(k p) c -> p k c", p=P))
                ps = psum_mm.tile([P, CH], f32, tag="ps")
                for k in range(KD):
                    nc.tensor.matmul(ps, lhsT=wt[:, k, :],
                                     rhs=xw_sb[:, k, HALO:],
                                     start=(k == 0), stop=(k == KD - 1))
                if flags["use_bq"]:
                    nc.scalar.activation(qT[:, m, :], ps,
                                         mybir.ActivationFunctionType.Identity,
                                         bias=bias_ap(I_BQ, m))
                else:
                    nc.scalar.copy(qT[:, m, :], ps)
            # kT [d, win] with elu(x)+1 = relu(x) + exp(min(x, 0))
            kT = ap_.tile([P, KD, WIN], f32r)
            wvts = []
            for m in range(KD):
                if m == KD - 1:
                    # first v-weight half streams in behind the wk tiles so
                    # the v-proj can start right as kT finishes
                    wvt0 = wv_pool.tile([P, KD, 512], f32r, tag="wv")
                    wvts.append(wvt0)
                    nc.sync.dma_start(
                        out=wvt0,
                        in_=wv.rearrange("(k p) n -> p k n", p=P)[:, :, :512])
                wt = wqk_pool.tile([P, KD, P], f32r, tag="wqk")
                nc.sync.dma_start(out=wt,
                                  in_=wk[m].rearrange("(k p) c -> p k c", p=P))
                for half in range(2):
                    sl = slice(320 * half, 320 * (half + 1))
                    ps = psum_mm.tile([P, 320], f32, tag="ps")
                    for k in range(KD):
                        nc.tensor.matmul(ps, lhsT=wt[:, k, :],
                                         rhs=xw_sb[:, k, sl],
                                         start=(k == 0), stop=(k == KD - 1))
                    bk_b = bias_ap(I_BK, m) if flags["use_bk"] else zero_b
                    a = lnt.tile([P, 320], f32, tag="elu")
                    nc.scalar.activation(a, ps,
                                         mybir.ActivationFunctionType.Relu,
                                         bias=bk_b)
                    mn = lnt.tile([P, 320], f32, tag="elu")
                    nc.vector.tensor_sub(mn, ps, a)
                    e = lnt.tile([P, 320], f32, tag="elu")
                    nc.scalar.activation(e, mn,
                                         mybir.ActivationFunctionType.Exp,
                                         bias=bk_b)
                    nc.vector.tensor_add(kT[:, m, sl], a, e)

            # v [win_tok, d] token-major
            vt = ap_.tile([P, MW, D], f32r)
            wvt1 = wv_pool.tile([P, KD, 512], f32r, tag="wv")
            wvts.append(wvt1)
            nc.sync.dma_start(
                out=wvt1,
                in_=wv.rearrange("(k p) n -> p k n", p=P)[:, :, 512:])
            mask_sb = ap_.tile([P, MW, CH], f32)
            nc.sync.dma_start(out=mask_sb,
                              in_=maskT.rearrange("(m p) q -> p m q", p=P))
            nc.sync.dma_start(out=sm, in_=smalls)
            nc.sync.dma_start(out=b1s, in_=b1v)
            for half in range(2):
                wvt = wvts[half]
                for m in range(MW):
                    ps = psum_mm.tile([P, CH], f32, tag="ps")
                    for k in range(KD):
                        nc.tensor.matmul(ps, lhsT=xw_sb[:, k, P * m:P * (m + 1)],
                                         rhs=wvt[:, k, :],
                                         start=(k == 0), stop=(k == KD - 1))
                    dst = vt[:, m, 512 * half:512 * (half + 1)]
                    if flags["use_bv"]:
                        nc.vector.tensor_add(dst, ps,
                                             bv_bc[:, 512 * half:512 * (half + 1)])
                    else:
                        nc.scalar.copy(dst, ps)

            # scoresT [win_tok, q] * maskT
            scoresM = ap_.tile([P, MW, CH], f32r)
            for m in range(MW):
                ps = psum_mm.tile([P, CH], f32, tag="ps")
                for k in range(KD):
                    nc.tensor.matmul(ps, lhsT=kT[:, k, P * m:P * (m + 1)],
                                     rhs=qT[:, k, :],
                                     start=(k == 0), stop=(k == KD - 1))
                nc.vector.tensor_mul(scoresM[:, m, :], ps, mask_sb[:, m, :])

            # attn_outT [d, q]  (reuses qT slot)
            at = ap_.tile([P, KD, CH], f32r, tag="qslot")
            for m in range(KD):
                ps = psum_mm.tile([P, CH], f32, tag="ps")
                for k in range(MW):
                    nc.tensor.matmul(ps, lhsT=vt[:, k, P * m:P * (m + 1)],
                                     rhs=scoresM[:, k, :],
                                     start=(k == 0), stop=(k == MW - 1))
                nc.scalar.copy(at[:, m, :], ps)

            # fastT + residual -> h, split in token halves; LN(A) overlaps
            # the B-half o-proj matmuls.
            wts = {}
            oseq = _halves_seq(KD, 2)
            for i, (t2, m) in enumerate(oseq):
                sl = tsl(t2)
                if t2 == 0:
                    wt = wqk_pool.tile([P, KD, P], f32r, tag="wqk")
                    nc.sync.dma_start(out=wt,
                                      in_=wo[m].rearrange("(k p) c -> p k c", p=P))
                    wts[m] = wt
                wt = wts[m]
                ps = psum_mm.tile([P, TT], f32, tag="ps")
                for k in range(KD):
                    nc.tensor.matmul(ps, lhsT=wt[:, k, :], rhs=at[:, k, sl],
                                     start=(k == 0), stop=(k == KD - 1))
                nc.vector.tensor_add(h[:, m, sl], ps, xw_sb[:, m, HALO + TT * t2:
                                                             HALO + TT * (t2 + 1)])
                if flags["use_bo"]:
                    nc.vector.tensor_scalar_add(h[:, m, sl], c(h[:, m, sl]),
                                                bias_ap(I_BO, m))
                if t2 == 0 and m == KD - 1:
                    layernorm_half(I_GF, I_BF, 0, flags["use_gbf"])
            layernorm_half(I_GF, I_BF, 1, flags["use_gbf"])

        # ---------------- MLP stack (fp8 DoubleRow + correction) ----------
        UP_SHIFT = 24
        DN_SHIFT = 4
        with tc.tile_pool(name="w1p", bufs=10) as w1_pool, \
             tc.tile_pool(name="w2p", bufs=6) as w2_pool, \
             tc.tile_pool(name="ubp", bufs=4) as ub_pool, \
             tc.tile_pool(name="up", bufs=1) as u_pool:
            preload = {}
            for l in range(L):
                uc = u_pool.tile([P, 2, KU, CH], fp8, tag="uc")
                # ---- up-proj (A leads B by UP_SHIFT m-chunks)
                wts = preload
                preload = {}
                for i, (t2, m) in enumerate(_halves_seq(KU, UP_SHIFT)):
                    sl = tsl(t2)
                    g, j = divmod(m, 4)
                    if t2 == 0 and j == 0 and g not in wts:
                        w4 = w1_pool.tile([P, 4, 2, KD, P], fp8, tag="w1t")
                        nc.sync.dma_start(out=w4, in_=w1c[l, g])
                        wts[g] = w4
                    wt = wts[g]
                    ps = psum_mm.tile([P, TT], f32, tag="ps")
                    for kk in range(KD // 2):
                        nc.tensor.matmul(ps,
                                         lhsT=wt[:, j, 1, 2 * kk:2 * kk + 2, :],
                                         rhs=hc[:, 0, 2 * kk:2 * kk + 2, sl],
                                         start=(kk == 0), stop=False,
                                         perf_mode=DRM)
                    for k in range(KD):
                        nc.tensor.matmul(ps, lhsT=wt[:, j, :, k, :],
                                         rhs=hc[:, :, k, sl],
                                         start=False, stop=(k == KD - 1),
                                         perf_mode=DRM)
                    ub = ub_pool.tile([P, TT], bf16, tag="ub")
                    nc.scalar.activation(ub, ps,
                                         mybir.ActivationFunctionType.Gelu,
                                         bias=(b1s[:, l, m:m + 1]
                                               if flags["use_b1"] else zero_b),
                                         scale=inv_sw)
                    fp8_pair(uc[:, 0, m, sl], uc[:, 1, m, sl], ub)
                # ---- down-proj (A leads B by DN_SHIFT; LN(A) after last A)
                # prefetch the next layer's first up-proj weight groups now:
                # the SP DMA queue is clear here, so these transfers run
                # during the down-pass instead of bunching at the boundary.
                if l + 1 < L:
                    for g in range(2):
                        w4 = w1_pool.tile([P, 4, 2, KD, P], fp8, tag="w1t")
                        nc.sync.dma_start(out=w4, in_=w1c[l + 1, g])
                        preload[g] = w4
                wts = {}
                dseq = _halves_seq(KD, DN_SHIFT)
                for i, (t2, m) in enumerate(dseq):
                    sl = tsl(t2)
                    if t2 == 0:
                        wt = w2_pool.tile([P, 2, KU, P], fp8, tag="w2t")
                        nc.sync.dma_start(out=wt, in_=w2c[l, m])
                        wts[m] = wt
                    wt = wts[m]
                    ps = psum_mm.tile([P, TT], f32, tag="ps")
                    for kk in range(KU // 2):
                        nc.tensor.matmul(ps, lhsT=wt[:, 1, 2 * kk:2 * kk + 2, :],
                                         rhs=uc[:, 0, 2 * kk:2 * kk + 2, sl],
                                         start=(kk == 0), stop=False,
                                         perf_mode=DRM)
                    for k in range(KU):
                        nc.tensor.matmul(ps, lhsT=wt[:, :, k, :],
                                         rhs=uc[:, :, k, sl],
                                         start=False, stop=(k == KU - 1),
                                         perf_mode=DRM)
                    with nc.allow_low_precision(reason="f32r=fp32 bits"):
                        nc.vector.scalar_tensor_tensor(
                            h[:, m, sl], ps, 1.0 / SW, c(h[:, m, sl]),
                            mybir.AluOpType.mult, mybir.AluOpType.add)
                    if flags["use_b2"]:
                        nc.vector.tensor_scalar_add(h[:, m, sl], c(h[:, m, sl]),
                                                    bias_ap(I_B2(l), m))
                    if t2 == 0 and m == KD - 1:
                        layernorm_half(I_GC(l), I_BE(l), 0, flags["use_gbc"])
                layernorm_half(I_GC(l), I_BE(l), 1, flags["use_gbc"])

        # ---------------- LM head (fp8 DoubleRow + correction) ------------
        with tc.tile_pool(name="whp", bufs=6) as wh_pool, \
             tc.tile_pool(name="outp", bufs=8) as out_pool, \
             tc.tile_pool(name="bhp", bufs=2) as bh_pool:
            for n in range(NVC):
                wht = wh_pool.tile([P, 2, KD, NV], fp8)
                nc.sync.dma_start(out=wht, in_=whc[n])

                if flags["use_bh"]:
                    bh_bc = bh_pool.tile([P, NV], f32)
                    src = bass.AP(tensor=bhv.tensor,
                                  offset=bhv.offset + NV * n * 4,
                                  ap=[[0, P], [4, NV]])
                    nc.sync.dma_start(out=bh_bc, in_=src)
                for m in range(4):
                    ps = psum_mm.tile([P, NV], f32, tag="ps")
                    for kk in range(KD // 2):
                        nc.tensor.matmul(
                            ps,
                            lhsT=hc[:, 0, 2 * kk:2 * kk + 2, P * m:P * (m + 1)],
                            rhs=wht[:, 1, 2 * kk:2 * kk + 2, :],
                            start=(kk == 0), stop=False, perf_mode=DRM)
                    for k in range(KD):
                        nc.tensor.matmul(
                            ps,
                            lhsT=hc[:, :, k, P * m:P * (m + 1)],
                            rhs=wht[:, :, k, :],
                            start=False, stop=(k == KD - 1), perf_mode=DRM)
                    ot = out_pool.tile([P, NV], bf16)
                    if flags["use_bh"]:
                        sc = out_pool.tile([P, NV], f32, tag="osc")
                        nc.scalar.activation(sc, ps,
                                             mybir.ActivationFunctionType.Identity,
                                             bias=zero_b, scale=inv_sw)
                        nc.vector.tensor_add(ot, sc, bh_bc)
                    else:
                        nc.scalar.activation(ot, ps,
                                             mybir.ActivationFunctionType.Identity,
                                             bias=zero_b, scale=inv_sw)
                    nc.sync.dma_start(out=out[P * m:P * (m + 1), NV * n:NV * (n + 1)],
                                      in_=ot)

    nc.compile()
    return nc


def _get_program(flags):
    key = tuple(sorted(flags.items()))
    if key not in _CACHE:
        _CACHE[key] = _build_program(flags)
    return _CACHE[key]


def _f8pair(w):
    """w (f32) -> (d8, w8) fp8 e4m3 blocks of SW*w: SW*w ~ w8 + d8."""
    ws = np.clip(w * SW, -240.0, 240.0)
    w8 = ws.astype(E4NP)
    d8 = (ws - w8.astype(np.float32)).astype(E4NP)
    return d8, w8


def kernel(x, Wq, bq, Wk, bk, Wv, bv, Wo, bo, decay_param, g_fast, b_fast,
           W1, b1, W2, b2, g_cms, beta_cms, Wh, bh):
    x = np.asarray(x, np.float32)
    Wq, Wk, Wv, Wo = (np.asarray(a, np.float32) for a in (Wq, Wk, Wv, Wo))
    bq, bk, bv, bo = (np.asarray(a, np.float32) for a in (bq, bk, bv, bo))
    g_fast, b_fast = np.asarray(g_fast, np.float32), np.asarray(b_fast, np.float32)
    W1, W2 = np.asarray(W1, np.float32), np.asarray(W2, np.float32)
    b1, b2 = np.asarray(b1, np.float32), np.asarray(b2, np.float32)
    g_cms, beta_cms = np.asarray(g_cms, np.float32), np.asarray(beta_cms, np.float32)
    Wh, bh = np.asarray(Wh, np.float32), np.asarray(bh, np.float32)
    decay = float(1.0 / (1.0 + np.exp(-np.float64(np.asarray(decay_param)))))
    if decay ** HALO > 1e-12:
        raise NotImplementedError(
            f"decay={decay} too close to 1 for banded attention (halo={HALO})")

    flags = {
        "use_bq": bool(np.any(bq)), "use_bk": bool(np.any(bk)),
        "use_bv": bool(np.any(bv)), "use_bo": bool(np.any(bo)),
        "use_b1": bool(np.any(b1)), "use_b2": bool(np.any(b2)),
        "use_bh": bool(np.any(bh)),
        "use_gbf": bool(np.any(g_fast != 1.0) or np.any(b_fast)),
        "use_gbc": bool(np.any(g_cms != 1.0) or np.any(beta_cms)),
    }
    nc = _get_program(flags)

    # host-side weight layout prep (shared by all cores)
    wq_h = np.ascontiguousarray(Wq.reshape(D, KD, P).transpose(1, 0, 2))
    wk_h = np.ascontiguousarray(Wk.reshape(D, KD, P).transpose(1, 0, 2))
    wo_h = np.ascontiguousarray(Wo.reshape(D, KD, P).transpose(1, 0, 2))

    # fp8 pairs, SBUF-layout (partition-major): [l, m, p, b, k, c]
    d1, w1_8 = _f8pair(W1)                      # [L, D, ED]
    a = w1_8.reshape(L, KD, P, KU, P).transpose(0, 3, 2, 1, 4)
    d = d1.reshape(L, KD, P, KU, P).transpose(0, 3, 2, 1, 4)
    w1c_h = np.stack([d, a], axis=3)                            # [L,KU,P,2,KD,P]
    w1c_h = np.ascontiguousarray(
        w1c_h.reshape(L, KU // 4, 4, P, 2, KD, P).transpose(0, 1, 3, 2, 4, 5, 6))
    d2, w2_8 = _f8pair(W2)                      # [L, ED, D]
    a = w2_8.reshape(L, KU, P, KD, P).transpose(0, 3, 2, 1, 4)
    d = d2.reshape(L, KU, P, KD, P).transpose(0, 3, 2, 1, 4)
    w2c_h = np.ascontiguousarray(np.stack([d, a], axis=3))      # [L,KD,P,2,KU,P]
    dh_, wh_8 = _f8pair(Wh)                     # [D, V]
    a = wh_8.reshape(KD, P, NVC, NV).transpose(2, 1, 0, 3)      # [n, p, k, nv]
    d = dh_.reshape(KD, P, NVC, NV).transpose(2, 1, 0, 3)
    whc_h = np.ascontiguousarray(np.stack([d, a], axis=2))      # [NVC,P,2,KD,NV]

    smalls = np.zeros((NS, D), np.float32)
    smalls[I_BQ], smalls[I_BK], smalls[I_BV], smalls[I_BO] = bq, bk, bv, bo
    smalls[I_GF], smalls[I_BF] = g_fast, b_fast
    for l in range(L):
        smalls[I_B2(l)], smalls[I_GC(l)], smalls[I_BE(l)] = b2[l], g_cms[l], beta_cms[l]
    smalls = np.ascontiguousarray(
        smalls.reshape(NS, KD, P).transpose(2, 0, 1))        # [P, NS, KD]
    b1_r = np.ascontiguousarray(
        b1.reshape(L, KU, P).transpose(2, 0, 1))             # [P, L, KU]

    shared = {"wq": wq_h, "wk": wk_h, "wo": wo_h, "wv": Wv,
              "w1c": w1c_h, "w2c": w2c_h, "whc": whc_h,
              "smalls": smalls, "b1v": b1_r,
              "onesc": np.ones((P, 1), np.float32),
              "onesr": np.ones((1, P), np.float32)}
    if flags["use_bh"]:
        shared["bhv"] = bh
    if flags["use_bv"]:
        shared["bvv"] = bv

    in_maps = []
    i_loc = np.arange(CH)[:, None]
    j_loc = np.arange(WIN)[None, :]
    for c in range(NCORES):
        bidx, start = divmod(c * CH, S)
        w0 = start - HALO
        xwin = np.zeros((WIN, D), np.float32)
        lo = max(w0, 0)
        xwin[lo - w0:, :] = x[bidx, lo:start + CH]
        xw_h = np.ascontiguousarray(xwin.T)  # [D, WIN]
        expo = (HALO + i_loc) - j_loc - 1
        valid = (expo >= 0) & (w0 + j_loc >= 0)
        maskq = np.where(valid,
                         np.power(np.float64(decay), np.maximum(expo, 0)),
                         0.0).astype(np.float32)          # [CH, WIN]
        mask_h = np.ascontiguousarray(maskq.T)            # [WIN, CH]
        in_maps.append({**shared, "xw": xw_h, "maskT": mask_h})

    res = run_bass_kernel_spmd(nc, in_maps, list(range(NCORES)), trace=TRACE)
    outs = [res.results[c]["out"].astype(np.float32) for c in range(NCORES)]
    full = np.concatenate(outs, axis=0).reshape(B, S, V)
    kernel.last_result = res
    return full


# revision 41
# speedup vs baseline: 1.0109x; 1.0109x over previous
"""Trainium2 Bass kernel for nn_HOPE_7275674599449.

Decay-masked fast-weight attention + 4-layer MLP stack + LM head,
data-parallel over 8 NeuronCores (512 tokens each, 128-token halo for
the decay-banded attention; decay^128 underflows fp32 so the banding
is numerically exact).

Per-core program (feature-major activations [d_partitions, tokens]):
  - q/k/v/o projections + scores + attn in f32r (fp22 on PE, 1 cyc/row)
  - MLP + LM head matmuls in fp8e4 DoubleRow (0.5 cyc/row, 2 k-tiles
    per instruction) with an error-compensated split:
        W@h = W8@h8 + (dW8@h8 + W8@dh8),  W8 = fp8(128*W), dW8 = fp8(128*W - W8)
    The two correction products share one DoubleRow instruction per
    k-tile, so the whole thing costs 12 slot-pairs per 8 k-tiles =
    0.75x bf16 while landing ~bf16 accuracy (measured rel ~3e-3).
  - LayerNorms: partition-dim reductions via ones-matmul on PE (f32r),
    per-token stats broadcast via ones-matmul, elementwise on DVE/ACT.
  - Token-half (A/B) software pipeline: the o-proj / MLP matmuls and
    each LayerNorm are split into 256-token halves and emitted in a
    shifted order, so the LN + fp8-conversion chain of one half runs
    on DVE/ACT/Pool while the PE crunches the other half.
"""

import sys

sys.path.insert(0, "/opt/trn_rl_repo")

from contextlib import ExitStack

import ml_dtypes
import numpy as np

import concourse.bass as bass
import concourse.tile as tile
from concourse import bacc, mybir
from concourse.bass_utils import run_bass_kernel_spmd

P = 128
B, S, D, L, V = 2, 2048, 1024, 4, 32000
ED = 4 * D              # MLP hidden
CH = 512                # tokens per core
TT = CH // 2            # token half
HALO = 128
WIN = HALO + CH         # 640
KD = D // P             # 8
KU = ED // P            # 32
MW = WIN // P           # 5 window token chunks
NV = 500                # head free-dim chunk
NVC = V // NV           # 64
NCORES = 8
EPS = 1e-5
SW = 128.0              # fp8 weight pre-scale (power of 2, exact)

f32 = mybir.dt.float32
f32r = mybir.dt.float32r
bf16 = mybir.dt.bfloat16
fp8 = mybir.dt.float8e4
DRM = mybir.MatmulPerfMode.DoubleRow
E4NP = ml_dtypes.float8_e4m3   # TRN e4m3 (max 240)

# smalls stacking indices (rows of the [18, D] f32 "smalls" tensor)
I_BQ, I_BK, I_BV, I_BO, I_GF, I_BF = 0, 1, 2, 3, 4, 5
def I_B2(l): return 6 + 3 * l
def I_GC(l): return 7 + 3 * l
def I_BE(l): return 8 + 3 * l
NS = 6 + 3 * L

TRACE = False          # set by test.py for profiled runs
_CACHE = {}


def _halves_seq(n, shift):
    """Emission order (half, m): A leads B by `shift` m-chunks."""
    seq = [(0, m) for m in range(min(shift, n))]
    for m in range(shift, n):
        seq.append((1, m - shift))
        seq.append((0, m))
    for m in range(max(0, n - shift), n):
        seq.append((1, m))
    return seq


def _build_program(flags):
    """Build the per-core Bass/Tile program. flags: dict of use_* booleans."""
    nc = bacc.Bacc("TRN2", target_bir_lowering=False, debug=False,
                   num_devices=NCORES)

    xw = nc.dram_tensor("xw", [D, WIN], f32r, kind="ExternalInput").ap()
    maskT = nc.dram_tensor("maskT", [WIN, CH], f32, kind="ExternalInput").ap()
    wq = nc.dram_tensor("wq", [KD, D, P], f32r, kind="ExternalInput").ap()
    wk = nc.dram_tensor("wk", [KD, D, P], f32r, kind="ExternalInput").ap()
    wo = nc.dram_tensor("wo", [KD, D, P], f32r, kind="ExternalInput").ap()
    wv = nc.dram_tensor("wv", [D, D], f32r, kind="ExternalInput").ap()
    onesc = nc.dram_tensor("onesc", [P, 1], f32r, kind="ExternalInput").ap()
    onesr = nc.dram_tensor("onesr", [1, P], f32r, kind="ExternalInput").ap()
    # fp8 weight pairs: b=0 -> d (fp8 of scaled residual), b=1 -> hi fp8
    w1c = nc.dram_tensor("w1c", [L, KU // 4, P, 4, 2, KD, P], fp8,
                         kind="ExternalInput").ap()
    w2c = nc.dram_tensor("w2c", [L, KD, P, 2, KU, P], fp8, kind="ExternalInput").ap()
    whc = nc.dram_tensor("whc", [NVC, P, 2, KD, NV], fp8, kind="ExternalInput").ap()
    smalls = nc.dram_tensor("smalls", [P, NS, KD], f32, kind="ExternalInput").ap()
    b1v = nc.dram_tensor("b1v", [P, L, KU], f32, kind="ExternalInput").ap()
    bhv = None
    if flags["use_bh"]:
        bhv = nc.dram_tensor("bhv", [V], f32, kind="ExternalInput").ap()
    bvv = None
    if flags["use_bv"]:
        bvv = nc.dram_tensor("bvv", [D], f32, kind="ExternalInput").ap()
    out = nc.dram_tensor("out", [CH, V], bf16, kind="ExternalOutput").ap()

    # f32r tiles: PE reads fp32 bits, truncates to fp22, 1 cyc/row (vs 4
    # for fp32) when the moving free dim is >=256. The BIR verifier wants
    # every producer of an f32r-matmul operand typed f32r, so the tiles are
    # declared f32r and elementwise engines read them via .bitcast(f32).
    def c(ap): return ap.bitcast(f32)

    def tsl(t2):
        return slice(TT * t2, TT * (t2 + 1))

    with tile.TileContext(nc) as tc, ExitStack() as ctx:
        persist = ctx.enter_context(tc.tile_pool(name="persist", bufs=1))
        sqp = ctx.enter_context(tc.tile_pool(name="sqp", bufs=6))
        lnt = ctx.enter_context(tc.tile_pool(name="lnt", bufs=4))
        psum_mm = ctx.enter_context(
            tc.tile_pool(name="psum_mm", bufs=5, space="PSUM"))
        psum_s = ctx.enter_context(
            tc.tile_pool(name="psum_s", bufs=2, space="PSUM"))
        psum_bc = ctx.enter_context(
            tc.tile_pool(name="psum_bc", bufs=1, space="PSUM"))

        h = persist.tile([P, KD, CH], f32r)
        # hc: fp8 pair of h. [:,0,k,:] = h8, [:,1,k,:] = dh8 = fp8(h - h8)
        hc = persist.tile([P, 2, KD, CH], fp8)
        sm = persist.tile([P, NS, KD], f32)
        b1s = persist.tile([P, L, KU], f32)
        ones_col = persist.tile([P, 1], f32r)
        ones_row = persist.tile([1, P], f32r)
        eps_t = persist.tile([1, 1], f32)
        nc.vector.memset(eps_t, EPS)
        zero_b = persist.tile([P, 1], f32)
        nc.vector.memset(zero_b, 0.0)
        inv_sw = persist.tile([P, 1], f32)
        nc.vector.memset(inv_sw, 1.0 / SW)


        def bias_ap(idx, k):
            return sm[:, idx, k:k + 1]

        def fp8_pair(dst8, dstd, src):
            """dst8 = fp8(src); dstd = fp8(src - dst8). DVE does the sub."""
            with nc.allow_low_precision(reason="fp8 pair for DoubleRow"):
                nc.gpsimd.tensor_copy(dst8, src)
                nc.vector.tensor_sub(dstd, src, dst8)

        def fp8_pair_pool(dst8, dstd, src):
            """Same, entirely on Pool: keeps the LN-critical DVE free and
            avoids a cross-engine sem hop between copy and sub."""
            with nc.allow_low_precision(reason="fp8 pair for DoubleRow"):
                nc.gpsimd.tensor_copy(dst8, src)
                nc.gpsimd.tensor_sub(dstd, src, dst8)

        def layernorm_half(g_idx, b_idx, t2, apply_gb):
            """h[:, :, half] = LN(h)*g + b over d; refresh hc half."""
            sl = tsl(t2)
            ps_s = psum_s.tile([1, TT], f32, tag="lnsum")
            for k in range(KD):
                nc.tensor.matmul(ps_s, lhsT=ones_col, rhs=h[:, k, sl],
                                 start=(k == 0), stop=(k == KD - 1))
            ps_q = psum_s.tile([1, TT], f32, tag="lnsum")
            for k in range(KD):
                sq = sqp.tile([P, TT], f32r, tag="sq")
                nc.scalar.square(sq, c(h[:, k, sl]))
                nc.tensor.matmul(ps_q, lhsT=ones_col, rhs=sq,
                                 start=(k == 0), stop=(k == KD - 1))
            mean = lnt.tile([1, TT], f32r, tag="lnstat")
            nc.scalar.mul(mean, ps_s, 1.0 / D)
            ex2 = lnt.tile([1, TT], f32, tag="lnstat")
            nc.scalar.mul(ex2, ps_q, 1.0 / D)
            var = lnt.tile([1, TT], f32, tag="lnstat")
            nc.vector.tensor_mul(var, c(mean), c(mean))
            nc.vector.tensor_sub(var, ex2, var)
            std = lnt.tile([1, TT], f32, tag="lnstat")
            nc.scalar.activation(std, var,
                                 mybir.ActivationFunctionType.Sqrt, bias=eps_t)
            rstd = lnt.tile([1, TT], f32r, tag="lnstat")
            with nc.allow_low_precision(reason="f32r carries full fp32 bits"):
                nc.vector.reciprocal(rstd, std)
            bc2 = psum_bc.tile([P, 2, TT], f32, tag="bc")
            ps_mb, ps_rb = bc2[:, 0, :], bc2[:, 1, :]
            nc.tensor.matmul(ps_mb, lhsT=ones_row, rhs=mean,
                             start=True, stop=True)
            nc.tensor.matmul(ps_rb, lhsT=ones_row, rhs=rstd,
                             start=True, stop=True)
            for k in range(KD):
                t = lnt.tile([P, TT], f32, tag="lntmp")
                nc.vector.tensor_sub(t, c(h[:, k, sl]), ps_mb)
                if apply_gb:
                    nc.vector.tensor_mul(t, t, ps_rb)
                    nc.scalar.activation(h[:, k, sl], t,
                                         mybir.ActivationFunctionType.Identity,
                                         bias=bias_ap(b_idx, k),
                                         scale=bias_ap(g_idx, k))
                else:
                    # g==1, b==0: the normalized value IS h
                    with nc.allow_low_precision(reason="f32r=fp32 bits"):
                        nc.vector.tensor_mul(h[:, k, sl], t, ps_rb)
                fp8_pair_pool(hc[:, 0, k, sl], hc[:, 1, k, sl],
                              c(h[:, k, sl]))

        # ---------------- attention ----------------
        with tc.tile_pool(name="attn", bufs=1) as ap_, \
             tc.tile_pool(name="wqk", bufs=6) as wqk_pool, \
             tc.tile_pool(name="wvp", bufs=2) as wv_pool:
            xw_sb = ap_.tile([P, KD, WIN], f32r)
            wq_r = [None] * KD
            # first q-proj weight tile before everything else: it gates the
            # very first matmul
            wt0 = wqk_pool.tile([P, KD, P], f32r, tag="wqk")
            nc.sync.dma_start(out=wt0,
                              in_=wq[0].rearrange("(k p) c -> p k c", p=P))
            for k in range(KD):
                nc.sync.dma_start(
                    out=xw_sb[:, k, :],
                    in_=xw.rearrange("(k p) t -> p k t", p=P)[:, k, :])
            nc.sync.dma_start(out=ones_col, in_=onesc)
            nc.sync.dma_start(out=ones_row, in_=onesr)
            if flags["use_bv"]:
                bv_bc = ap_.tile([P, D], f32)
                src = bass.AP(tensor=bvv.tensor, offset=bvv.offset,
                              ap=[[0, P], bvv.ap[0]])
                nc.sync.dma_start(out=bv_bc, in_=src)

            # qT [d, q]
            qT = ap_.tile([P, KD, CH], f32r, tag="qslot")
            for m in range(KD):
                if m == 0:
                    wt = wt0
                else:
                    wt = wqk_pool.tile([P, KD, P], f32r, tag="wqk")
                    nc.sync.dma_start(out=wt,
                                      in_=wq[m].rearrange("(k p) c -> p k c", p=P))
                ps = psum_mm.tile([P, CH], f32, tag="ps")
                for k in range(KD):
                    nc.tensor.matmul(ps, lhsT=wt[:, k, :],
                                     rhs=xw_sb[:, k, HALO:],
                                     start=(k == 0), stop=(k == KD - 1))
                if flags["use_bq"]:
                    nc.scalar.activation(qT[:, m, :], ps,
                                         mybir.ActivationFunctionType.Identity,
                                         bias=bias_ap(I_BQ, m))
                else:
                    nc.scalar.copy(qT[:, m, :], ps)
            # kT [d, win] with elu(x)+1 = relu(x) + exp(min(x, 0))
            kT = ap_.tile([P, KD, WIN], f32r)
            wvts = []
            for m in range(KD):
                if m == KD - 1:
                    # first v-weight half streams in behind the wk tiles so
                    # the v-proj can start right as kT finishes
                    wvt0 = wv_pool.tile([P, KD, 512], f32r, tag="wv")
                    wvts.append(wvt0)
                    nc.sync.dma_start(
                        out=wvt0,
                        in_=wv.rearrange("(k p) n -> p k n", p=P)[:, :, :512])
                wt = wqk_pool.tile([P, KD, P], f32r, tag="wqk")
                nc.sync.dma_start(out=wt,
                                  in_=wk[m].rearrange("(k p) c -> p k c", p=P))
                for half in range(2):
                    sl = slice(320 * half, 320 * (half + 1))
                    ps = psum_mm.tile([P, 320], f32, tag="ps")
                    for k in range(KD):
                        nc.tensor.matmul(ps, lhsT=wt[:, k, :],
                                         rhs=xw_sb[:, k, sl],
                                         start=(k == 0), stop=(k == KD - 1))
                    bk_b = bias_ap(I_BK, m) if flags["use_bk"] else zero_b
                    a = lnt.tile([P, 320], f32, tag="elu")
                    nc.scalar.activation(a, ps,
                                         mybir.ActivationFunctionType.Relu,
                                         bias=bk_b)
                    mn = lnt.tile([P, 320], f32, tag="elu")
                    nc.vector.tensor_sub(mn, ps, a)
                    e = lnt.tile([P, 320], f32, tag="elu")
                    nc.scalar.activation(e, mn,
                                         mybir.ActivationFunctionType.Exp,
                                         bias=bk_b)
                    nc.vector.tensor_add(kT[:, m, sl], a, e)

            # v [win_tok, d] token-major
            vt = ap_.tile([P, MW, D], f32r)
            wvt1 = wv_pool.tile([P, KD, 512], f32r, tag="wv")
            wvts.append(wvt1)
            nc.sync.dma_start(
                out=wvt1,
                in_=wv.rearrange("(k p) n -> p k n", p=P)[:, :, 512:])
            mask_sb = ap_.tile([P, MW, CH], f32)
            nc.sync.dma_start(out=mask_sb,
                              in_=maskT.rearrange("(m p) q -> p m q", p=P))
            nc.sync.dma_start(out=sm, in_=smalls)
            nc.sync.dma_start(out=b1s, in_=b1v)
            for half in range(2):
                wvt = wvts[half]
                for m in range(MW):
                    ps = psum_mm.tile([P, CH], f32, tag="ps")
                    for k in range(KD):
                        nc.tensor.matmul(ps, lhsT=xw_sb[:, k, P * m:P * (m + 1)],
                                         rhs=wvt[:, k, :],
                                         start=(k == 0), stop=(k == KD - 1))
                    dst = vt[:, m, 512 * half:512 * (half + 1)]
                    if flags["use_bv"]:
                        nc.vector.tensor_add(dst, ps,
                                             bv_bc[:, 512 * half:512 * (half + 1)])
                    else:
                        nc.scalar.copy(dst, ps)

            # scoresT [win_tok, q] * maskT
            scoresM = ap_.tile([P, MW, CH], f32r)
            for m in range(MW):
                ps = psum_mm.tile([P, CH], f32, tag="ps")
                for k in range(KD):
                    nc.tensor.matmul(ps, lhsT=kT[:, k, P * m:P * (m + 1)],
                                     rhs=qT[:, k, :],
                                     start=(k == 0), stop=(k == KD - 1))
                nc.vector.tensor_mul(scoresM[:, m, :], ps, mask_sb[:, m, :])

            # attn_outT [d, q]  (reuses qT slot)
            at = ap_.tile([P, KD, CH], f32r, tag="qslot")
            for m in range(KD):
                ps = psum_mm.tile([P, CH], f32, tag="ps")
                for k in range(MW):
                    nc.tensor.matmul(ps, lhsT=vt[:, k, P * m:P * (m + 1)],
                                     rhs=scoresM[:, k, :],
                                     start=(k == 0), stop=(k == MW - 1))
                nc.scalar.copy(at[:, m, :], ps)

            # fastT + residual -> h, split in token halves; LN(A) overlaps
            # the B-half o-proj matmuls.
            wts = {}
            oseq = _halves_seq(KD, 2)
            for i, (t2, m) in enumerate(oseq):
                sl = tsl(t2)
                if t2 == 0:
                    wt = wqk_pool.tile([P, KD, P], f32r, tag="wqk")
                    nc.sync.dma_start(out=wt,
                                      in_=wo[m].rearrange("(k p) c -> p k c", p=P))
                    wts[m] = wt
                wt = wts[m]
                ps = psum_mm.tile([P, TT], f32, tag="ps")
                for k in range(KD):
                    nc.tensor.matmul(ps, lhsT=wt[:, k, :], rhs=at[:, k, sl],
                                     start=(k == 0), stop=(k == KD - 1))
                nc.vector.tensor_add(h[:, m, sl], ps, xw_sb[:, m, HALO + TT * t2:
                                                             HALO + TT * (t2 + 1)])
                if flags["use_bo"]:
                    nc.vector.tensor_scalar_add(h[:, m, sl], c(h[:, m, sl]),
                                                bias_ap(I_BO, m))
                if t2 == 0 and m == KD - 1:
                    layernorm_half(I_GF, I_BF, 0, flags["use_gbf"])
            layernorm_half(I_GF, I_BF, 1, flags["use_gbf"])

        # ---------------- MLP stack (fp8 DoubleRow + correction) ----------
        UP_SHIFT = 24
        DN_SHIFT = 4
        with tc.tile_pool(name="w1p", bufs=10) as w1_pool, \
             tc.tile_pool(name="w2p", bufs=6) as w2_pool, \
             tc.tile_pool(name="ubp", bufs=6) as ub_pool, \
             tc.tile_pool(name="up", bufs=1) as u_pool:
            preload = {}
            for l in range(L):
                uc = u_pool.tile([P, 2, KU, CH], fp8, tag="uc")
                # ---- up-proj (A leads B by UP_SHIFT m-chunks)
                wts = preload
                preload = {}
                for i, (t2, m) in enumerate(_halves_seq(KU, UP_SHIFT)):
                    sl = tsl(t2)
                    g, j = divmod(m, 4)
                    if t2 == 0 and j == 0 and g not in wts:
                        w4 = w1_pool.tile([P, 4, 2, KD, P], fp8, tag="w1t")
                        nc.sync.dma_start(out=w4, in_=w1c[l, g])
                        wts[g] = w4
                    wt = wts[g]
                    ps = psum_mm.tile([P, TT], f32, tag="ps")
                    for kk in range(KD // 2):
                        nc.tensor.matmul(ps,
                                         lhsT=wt[:, j, 1, 2 * kk:2 * kk + 2, :],
                                         rhs=hc[:, 0, 2 * kk:2 * kk + 2, sl],
                                         start=(kk == 0), stop=False,
                                         perf_mode=DRM)
                    for k in range(KD):
                        nc.tensor.matmul(ps, lhsT=wt[:, j, :, k, :],
                                         rhs=hc[:, :, k, sl],
                                         start=False, stop=(k == KD - 1),
                                         perf_mode=DRM)
                    ub = ub_pool.tile([P, TT], bf16, tag="ub")
                    nc.scalar.activation(ub, ps,
                                         mybir.ActivationFunctionType.Gelu,
                                         bias=(b1s[:, l, m:m + 1]
                                               if flags["use_b1"] else zero_b),
                                         scale=inv_sw)
                    fp8_pair(uc[:, 0, m, sl], uc[:, 1, m, sl], ub)
                # ---- down-proj (A leads B by DN_SHIFT; LN(A) after last A)
                # prefetch the next layer's first up-proj weight groups now:
                # the SP DMA queue is clear here, so these transfers run
                # during the down-pass instead of bunching at the boundary.
                if l + 1 < L:
                    for g in range(2):
                        w4 = w1_pool.tile([P, 4, 2, KD, P], fp8, tag="w1t")
                        nc.sync.dma_start(out=w4, in_=w1c[l + 1, g])
                        preload[g] = w4
                wts = {}
                dseq = _halves_seq(KD, DN_SHIFT)
                for i, (t2, m) in enumerate(dseq):
                    sl = tsl(t2)
                    if t2 == 0:
                        wt = w2_pool.tile([P, 2, KU, P], fp8, tag="w2t")
                        nc.sync.dma_start(out=wt, in_=w2c[l, m])
                        wts[m] = wt
                    wt = wts[m]
                    ps = psum_mm.tile([P, TT], f32, tag="ps")
                    for kk in range(KU // 2):
                        nc.tensor.matmul(ps, lhsT=wt[:, 1, 2 * kk:2 * kk + 2, :],
                                         rhs=uc[:, 0, 2 * kk:2 * kk + 2, sl],
                                         start=(kk == 0), stop=False,
                                         perf_mode=DRM)
                    for k in range(KU):
                        nc.tensor.matmul(ps, lhsT=wt[:, :, k, :],
                                         rhs=uc[:, :, k, sl],
                                         start=False, stop=(k == KU - 1),
                                         perf_mode=DRM)
                    with nc.allow_low_precision(reason="f32r=fp32 bits"):
                        nc.vector.scalar_tensor_tensor(
                            h[:, m, sl], ps, 1.0 / SW, c(h[:, m, sl]),
                            mybir.AluOpType.mult, mybir.AluOpType.add)
                    if flags["use_b2"]:
                        nc.vector.tensor_scalar_add(h[:, m, sl], c(h[:, m, sl]),
                                                    bias_ap(I_B2(l), m))
                    if t2 == 0 and m == KD - 1:
                        layernorm_half(I_GC(l), I_BE(l), 0, flags["use_gbc"])
                layernorm_half(I_GC(l), I_BE(l), 1, flags["use_gbc"])

        # ---------------- LM head (fp8 DoubleRow + correction) ------------
        with tc.tile_pool(name="whp", bufs=10) as wh_pool, \
             tc.tile_pool(name="outp", bufs=12) as out_pool, \
             tc.tile_pool(name="bhp", bufs=2) as bh_pool:
            for n in range(NVC):
                wht = wh_pool.tile([P, 2, KD, NV], fp8)
                nc.sync.dma_start(out=wht, in_=whc[n])

                if flags["use_bh"]:
                    bh_bc = bh_pool.tile([P, NV], f32)
                    src = bass.AP(tensor=bhv.tensor,
                                  offset=bhv.offset + NV * n * 4,
                                  ap=[[0, P], [4, NV]])
                    nc.sync.dma_start(out=bh_bc, in_=src)
                for m in range(4):
                    ps = psum_mm.tile([P, NV], f32, tag="ps")
                    for kk in range(KD // 2):
                        nc.tensor.matmul(
                            ps,
                            lhsT=hc[:, 0, 2 * kk:2 * kk + 2, P * m:P * (m + 1)],
                            rhs=wht[:, 1, 2 * kk:2 * kk + 2, :],
                            start=(kk == 0), stop=False, perf_mode=DRM)
                    for k in range(KD):
                        nc.tensor.matmul(
                            ps,
                            lhsT=hc[:, :, k, P * m:P * (m + 1)],
                            rhs=wht[:, :, k, :],
                            start=False, stop=(k == KD - 1), perf_mode=DRM)
                    ot = out_pool.tile([P, NV], bf16)
                    if flags["use_bh"]:
                        sc = out_pool.tile([P, NV], f32, tag="osc")
                        nc.scalar.activation(sc, ps,
                                             mybir.ActivationFunctionType.Identity,
                                             bias=zero_b, scale=inv_sw)
                        nc.vector.tensor_add(ot, sc, bh_bc)
                    else:
                        nc.scalar.activation(ot, ps,
                                             mybir.ActivationFunctionType.Identity,
                                             bias=zero_b, scale=inv_sw)
                    nc.sync.dma_start(out=out[P * m:P * (m + 1), NV * n:NV * (n + 1)],
                                      in_=ot)

    nc.compile()
    return nc


def _get_program(flags):
    key = tuple(sorted(flags.items()))
    if key not in _CACHE:
        _CACHE[key] = _build_program(flags)
    return _CACHE[key]


def _f8pair(w):
    """w (f32) -> (d8, w8) fp8 e4m3 blocks of SW*w: SW*w ~ w8 + d8."""
    ws = np.clip(w * SW, -240.0, 240.0)
    w8 = ws.astype(E4NP)
    d8 = (ws - w8.astype(np.float32)).astype(E4NP)
    return d8, w8


def kernel(x, Wq, bq, Wk, bk, Wv, bv, Wo, bo, decay_param, g_fast, b_fast,
           W1, b1, W2, b2, g_cms, beta_cms, Wh, bh):
    x = np.asarray(x, np.float32)
    Wq, Wk, Wv, Wo = (np.asarray(a, np.float32) for a in (Wq, Wk, Wv, Wo))
    bq, bk, bv, bo = (np.asarray(a, np.float32) for a in (bq, bk, bv, bo))
    g_fast, b_fast = np.asarray(g_fast, np.float32), np.asarray(b_fast, np.float32)
    W1, W2 = np.asarray(W1, np.float32), np.asarray(W2, np.float32)
    b1, b2 = np.asarray(b1, np.float32), np.asarray(b2, np.float32)
    g_cms, beta_cms = np.asarray(g_cms, np.float32), np.asarray(beta_cms, np.float32)
    Wh, bh = np.asarray(Wh, np.float32), np.asarray(bh, np.float32)
    decay = float(1.0 / (1.0 + np.exp(-np.float64(np.asarray(decay_param)))))
    if decay ** HALO > 1e-12:
        raise NotImplementedError(
            f"decay={decay} too close to 1 for banded attention (halo={HALO})")

    flags = {
        "use_bq": bool(np.any(bq)), "use_bk": bool(np.any(bk)),
        "use_bv": bool(np.any(bv)), "use_bo": bool(np.any(bo)),
        "use_b1": bool(np.any(b1)), "use_b2": bool(np.any(b2)),
        "use_bh": bool(np.any(bh)),
        "use_gbf": bool(np.any(g_fast != 1.0) or np.any(b_fast)),
        "use_gbc": bool(np.any(g_cms != 1.0) or np.any(beta_cms)),
    }
    nc = _get_program(flags)

    # host-side weight layout prep (shared by all cores)
    wq_h = np.ascontiguousarray(Wq.reshape(D, KD, P).transpose(1, 0, 2))
    wk_h = np.ascontiguousarray(Wk.reshape(D, KD, P).transpose(1, 0, 2))
    wo_h = np.ascontiguousarray(Wo.reshape(D, KD, P).transpose(1, 0, 2))

    # fp8 pairs, SBUF-layout (partition-major): [l, m, p, b, k, c]
    d1, w1_8 = _f8pair(W1)                      # [L, D, ED]
    a = w1_8.reshape(L, KD, P, KU, P).transpose(0, 3, 2, 1, 4)
    d = d1.reshape(L, KD, P, KU, P).transpose(0, 3, 2, 1, 4)
    w1c_h = np.stack([d, a], axis=3)                            # [L,KU,P,2,KD,P]
    w1c_h = np.ascontiguousarray(
        w1c_h.reshape(L, KU // 4, 4, P, 2, KD, P).transpose(0, 1, 3, 2, 4, 5, 6))
    d2, w2_8 = _f8pair(W2)                      # [L, ED, D]
    a = w2_8.reshape(L, KU, P, KD, P).transpose(0, 3, 2, 1, 4)
    d = d2.reshape(L, KU, P, KD, P).transpose(0, 3, 2, 1, 4)
    w2c_h = np.ascontiguousarray(np.stack([d, a], axis=3))      # [L,KD,P,2,KU,P]
    dh_, wh_8 = _f8pair(Wh)                     # [D, V]
    a = wh_8.reshape(KD, P, NVC, NV).transpose(2, 1, 0, 3)      # [n, p, k, nv]
    d = dh_.reshape(KD, P, NVC, NV).transpose(2, 1, 0, 3)
    whc_h = np.ascontiguousarray(np.stack([d, a], axis=2))      # [NVC,P,2,KD,NV]

    smalls = np.zeros((NS, D), np.float32)
    smalls[I_BQ], smalls[I_BK], smalls[I_BV], smalls[I_BO] = bq, bk, bv, bo
    smalls[I_GF], smalls[I_BF] = g_fast, b_fast
    for l in range(L):
        smalls[I_B2(l)], smalls[I_GC(l)], smalls[I_BE(l)] = b2[l], g_cms[l], beta_cms[l]
    smalls = np.ascontiguousarray(
        smalls.reshape(NS, KD, P).transpose(2, 0, 1))        # [P, NS, KD]
    b1_r = np.ascontiguousarray(
        b1.reshape(L, KU, P).transpose(2, 0, 1))             # [P, L, KU]

    shared = {"wq": wq_h, "wk": wk_h, "wo": wo_h, "wv": Wv,
              "w1c": w1c_h, "w2c": w2c_h, "whc": whc_h,
              "smalls": smalls, "b1v": b1_r,
              "onesc": np.ones((P, 1), np.float32),
              "onesr": np.ones((1, P), np.float32)}
    if flags["use_bh"]:
        shared["bhv"] = bh
    if flags["use_bv"]:
        shared["bvv"] = bv

    in_maps = []
    i_loc = np.arange(CH)[:, None]
    j_loc = np.arange(WIN)[None, :]
    for c in range(NCORES):
        bidx, start = divmod(c * CH, S)
        w0 = start - HALO
        xwin = np.zeros((WIN, D), np.float32)
        lo = max(w0, 0)
        xwin[lo - w0:, :] = x[bidx, lo:start + CH]
        xw_h = np.ascontiguousarray(xwin.T)  # [D, WIN]
        expo = (HALO + i_loc) - j_loc - 1
        valid = (expo >= 0) & (w0 + j_loc >= 0)
        maskq = np.where(valid,
                         np.power(np.float64(decay), np.maximum(expo, 0)),
                         0.0).astype(np.float32)          # [CH, WIN]
        mask_h = np.ascontiguousarray(maskq.T)            # [WIN, CH]
        in_maps.append({**shared, "xw": xw_h, "maskT": mask_h})

    res = run_bass_kernel_spmd(nc, in_maps, list(range(NCORES)), trace=TRACE)
    outs = [res.results[c]["out"].astype(np.float32) for c in range(NCORES)]
    full = np.concatenate(outs, axis=0).reshape(B, S, V)
    kernel.last_result = res
    return full


# revision 56
# speedup vs baseline: 1.0148x; 1.0039x over previous
"""Trainium2 Bass kernel for nn_HOPE_7275674599449.

Decay-masked fast-weight attention + 4-layer MLP stack + LM head,
data-parallel over 8 NeuronCores (512 tokens each, 128-token halo for
the decay-banded attention; decay^128 underflows fp32 so the banding
is numerically exact).

Per-core program (feature-major activations [d_partitions, tokens]):
  - q/k/v/o projections + scores + attn in f32r (fp22 on PE, 1 cyc/row)
  - MLP + LM head matmuls in fp8e4 DoubleRow (0.5 cyc/row, 2 k-tiles
    per instruction) with an error-compensated split:
        W@h = W8@h8 + (dW8@h8 + W8@dh8),  W8 = fp8(128*W), dW8 = fp8(128*W - W8)
    The two correction products share one DoubleRow instruction per
    k-tile, so the whole thing costs 12 slot-pairs per 8 k-tiles =
    0.75x bf16 while landing ~bf16 accuracy (measured rel ~3e-3).
  - LayerNorms: partition-dim reductions via ones-matmul on PE (f32r),
    per-token stats broadcast via ones-matmul, elementwise on DVE/ACT.
  - Token-half (A/B) software pipeline: the o-proj / MLP matmuls and
    each LayerNorm are split into 256-token halves and emitted in a
    shifted order, so the LN + fp8-conversion chain of one half runs
    on DVE/ACT/Pool while the PE crunches the other half.
"""

import sys

sys.path.insert(0, "/opt/trn_rl_repo")

from contextlib import ExitStack

import ml_dtypes
import numpy as np

import concourse.bass as bass
import concourse.tile as tile
from concourse import bacc, mybir
from concourse.bass_utils import run_bass_kernel_spmd

P = 128
B, S, D, L, V = 2, 2048, 1024, 4, 32000
ED = 4 * D              # MLP hidden
CH = 512                # tokens per core
TT = CH // 2            # token half
HALO = 128
WIN = HALO + CH         # 640
KD = D // P             # 8
KU = ED // P            # 32
MW = WIN // P           # 5 window token chunks
NV = 500                # head free-dim chunk
NVC = V // NV           # 64
NCORES = 8
EPS = 1e-5
SW = 128.0              # fp8 weight pre-scale (power of 2, exact)

f32 = mybir.dt.float32
f32r = mybir.dt.float32r
bf16 = mybir.dt.bfloat16
fp8 = mybir.dt.float8e4
DRM = mybir.MatmulPerfMode.DoubleRow
E4NP = ml_dtypes.float8_e4m3   # TRN e4m3 (max 240)

# smalls stacking indices (rows of the [18, D] f32 "smalls" tensor)
I_BQ, I_BK, I_BV, I_BO, I_GF, I_BF = 0, 1, 2, 3, 4, 5
def I_B2(l): return 6 + 3 * l
def I_GC(l): return 7 + 3 * l
def I_BE(l): return 8 + 3 * l
NS = 6 + 3 * L

TRACE = False          # set by test.py for profiled runs
_CACHE = {}


def _halves_seq(n, shift):
    """Emission order (half, m): A leads B by `shift` m-chunks."""
    seq = [(0, m) for m in range(min(shift, n))]
    for m in range(shift, n):
        seq.append((1, m - shift))
        seq.append((0, m))
    for m in range(max(0, n - shift), n):
        seq.append((1, m))
    return seq


def _build_program(flags):
    """Build the per-core Bass/Tile program. flags: dict of use_* booleans."""
    nc = bacc.Bacc("TRN2", target_bir_lowering=False, debug=False,
                   num_devices=NCORES)

    xw = nc.dram_tensor("xw", [D, WIN], f32r, kind="ExternalInput").ap()
    maskT = nc.dram_tensor("maskT", [WIN, CH], f32, kind="ExternalInput").ap()
    wq = nc.dram_tensor("wq", [KD, D, P], f32r, kind="ExternalInput").ap()
    wk = nc.dram_tensor("wk", [KD, D, P], f32r, kind="ExternalInput").ap()
    wo = nc.dram_tensor("wo", [KD, D, P], f32r, kind="ExternalInput").ap()
    wv = nc.dram_tensor("wv", [D, D], f32r, kind="ExternalInput").ap()
    onesc = nc.dram_tensor("onesc", [P, 1], f32r, kind="ExternalInput").ap()
    onesr = nc.dram_tensor("onesr", [1, P], f32r, kind="ExternalInput").ap()
    # fp8 weight pairs: b=0 -> d (fp8 of scaled residual), b=1 -> hi fp8
    w1c = nc.dram_tensor("w1c", [L, KU // 4, P, 4, 2, KD, P], fp8,
                         kind="ExternalInput").ap()
    w2c = nc.dram_tensor("w2c", [L, KD, P, 2, KU, P], fp8, kind="ExternalInput").ap()
    whc = nc.dram_tensor("whc", [NVC, P, 2, KD, NV], fp8, kind="ExternalInput").ap()
    smalls = nc.dram_tensor("smalls", [P, NS, KD], f32, kind="ExternalInput").ap()
    b1v = nc.dram_tensor("b1v", [P, L, KU], f32, kind="ExternalInput").ap()
    bhv = None
    if flags["use_bh"]:
        bhv = nc.dram_tensor("bhv", [V], f32, kind="ExternalInput").ap()
    bvv = None
    if flags["use_bv"]:
        bvv = nc.dram_tensor("bvv", [D], f32, kind="ExternalInput").ap()
    out = nc.dram_tensor("out", [CH, V], bf16, kind="ExternalOutput").ap()

    # f32r tiles: PE reads fp32 bits, truncates to fp22, 1 cyc/row (vs 4
    # for fp32) when the moving free dim is >=256. The BIR verifier wants
    # every producer of an f32r-matmul operand typed f32r, so the tiles are
    # declared f32r and elementwise engines read them via .bitcast(f32).
    def c(ap): return ap.bitcast(f32)

    def tsl(t2):
        return slice(TT * t2, TT * (t2 + 1))

    with tile.TileContext(nc) as tc, ExitStack() as ctx:
        persist = ctx.enter_context(tc.tile_pool(name="persist", bufs=1))
        sqp = ctx.enter_context(tc.tile_pool(name="sqp", bufs=6))
        lnt = ctx.enter_context(tc.tile_pool(name="lnt", bufs=4))
        psum_mm = ctx.enter_context(
            tc.tile_pool(name="psum_mm", bufs=5, space="PSUM"))
        psum_s = ctx.enter_context(
            tc.tile_pool(name="psum_s", bufs=2, space="PSUM"))
        psum_bc = ctx.enter_context(
            tc.tile_pool(name="psum_bc", bufs=1, space="PSUM"))

        h = persist.tile([P, KD, CH], f32r)
        # hc: fp8 pair of h. [:,0,k,:] = h8, [:,1,k,:] = dh8 = fp8(h - h8)
        hc = persist.tile([P, 2, KD, CH], fp8)
        sm = persist.tile([P, NS, KD], f32)
        b1s = persist.tile([P, L, KU], f32)
        ones_col = persist.tile([P, 1], f32r)
        ones_row = persist.tile([1, P], f32r)
        eps_t = persist.tile([1, 1], f32)
        nc.vector.memset(eps_t, EPS)
        zero_b = persist.tile([P, 1], f32)
        nc.vector.memset(zero_b, 0.0)
        inv_sw = persist.tile([P, 1], f32)
        nc.vector.memset(inv_sw, 1.0 / SW)


        def bias_ap(idx, k):
            return sm[:, idx, k:k + 1]

        def fp8_pair(dst8, dstd, src):
            """dst8 = fp8(src); dstd = fp8(src - dst8). DVE does the sub."""
            with nc.allow_low_precision(reason="fp8 pair for DoubleRow"):
                nc.gpsimd.tensor_copy(dst8, src)
                nc.vector.tensor_sub(dstd, src, dst8)

        def fp8_pair_pool(dst8, dstd, src):
            """Same, entirely on Pool: keeps the LN-critical DVE free and
            avoids a cross-engine sem hop between copy and sub."""
            with nc.allow_low_precision(reason="fp8 pair for DoubleRow"):
                nc.gpsimd.tensor_copy(dst8, src)
                nc.gpsimd.tensor_sub(dstd, src, dst8)

        def layernorm_half(g_idx, b_idx, t2, apply_gb):
            """h[:, :, half] = LN(h)*g + b over d; refresh hc half."""
            sl = tsl(t2)
            ps_s = psum_s.tile([1, TT], f32, tag="lnsum")
            for k in range(KD):
                nc.tensor.matmul(ps_s, lhsT=ones_col, rhs=h[:, k, sl],
                                 start=(k == 0), stop=(k == KD - 1))
            ps_q = psum_s.tile([1, TT], f32, tag="lnsum")
            for k in range(KD):
                sq = sqp.tile([P, TT], f32r, tag="sq")
                nc.scalar.square(sq, c(h[:, k, sl]))
                nc.tensor.matmul(ps_q, lhsT=ones_col, rhs=sq,
                                 start=(k == 0), stop=(k == KD - 1))
            mean = lnt.tile([1, TT], f32r, tag="lnstat")
            nc.scalar.mul(mean, ps_s, 1.0 / D)
            ex2 = lnt.tile([1, TT], f32, tag="lnstat")
            nc.scalar.mul(ex2, ps_q, 1.0 / D)
            var = lnt.tile([1, TT], f32, tag="lnstat")
            nc.vector.tensor_mul(var, c(mean), c(mean))
            nc.vector.tensor_sub(var, ex2, var)
            std = lnt.tile([1, TT], f32, tag="lnstat")
            nc.scalar.activation(std, var,
                                 mybir.ActivationFunctionType.Sqrt, bias=eps_t)
            rstd = lnt.tile([1, TT], f32r, tag="lnstat")
            with nc.allow_low_precision(reason="f32r carries full fp32 bits"):
                nc.vector.reciprocal(rstd, std)
            bc2 = psum_bc.tile([P, 2, TT], f32, tag="bc")
            ps_mb, ps_rb = bc2[:, 0, :], bc2[:, 1, :]
            nc.tensor.matmul(ps_mb, lhsT=ones_row, rhs=mean,
                             start=True, stop=True)
            nc.tensor.matmul(ps_rb, lhsT=ones_row, rhs=rstd,
                             start=True, stop=True)
            for k in range(KD):
                t = lnt.tile([P, TT], f32, tag="lntmp")
                nc.vector.tensor_sub(t, c(h[:, k, sl]), ps_mb)
                if apply_gb:
                    nc.vector.tensor_mul(t, t, ps_rb)
                    nc.scalar.activation(h[:, k, sl], t,
                                         mybir.ActivationFunctionType.Identity,
                                         bias=bias_ap(b_idx, k),
                                         scale=bias_ap(g_idx, k))
                else:
                    # g==1, b==0: the normalized value IS h
                    with nc.allow_low_precision(reason="f32r=fp32 bits"):
                        nc.vector.tensor_mul(h[:, k, sl], t, ps_rb)
                fp8_pair_pool(hc[:, 0, k, sl], hc[:, 1, k, sl],
                              c(h[:, k, sl]))

        # ---------------- attention ----------------
        with tc.tile_pool(name="attn", bufs=1) as ap_, \
             tc.tile_pool(name="wqk", bufs=6) as wqk_pool, \
             tc.tile_pool(name="wvp", bufs=2) as wv_pool:
            xw_sb = ap_.tile([P, KD, WIN], f32r)
            wq_r = [None] * KD
            # first q-proj weight tile before everything else: it gates the
            # very first matmul
            wt0 = wqk_pool.tile([P, KD, P], f32r, tag="wqk")
            nc.sync.dma_start(out=wt0,
                              in_=wq[0].rearrange("(k p) c -> p k c", p=P))
            for k in range(KD):
                nc.sync.dma_start(
                    out=xw_sb[:, k, :],
                    in_=xw.rearrange("(k p) t -> p k t", p=P)[:, k, :])
            nc.sync.dma_start(out=ones_col, in_=onesc)
            nc.sync.dma_start(out=ones_row, in_=onesr)
            if flags["use_bv"]:
                bv_bc = ap_.tile([P, D], f32)
                src = bass.AP(tensor=bvv.tensor, offset=bvv.offset,
                              ap=[[0, P], bvv.ap[0]])
                nc.sync.dma_start(out=bv_bc, in_=src)

            # qT [d, q]
            qT = ap_.tile([P, KD, CH], f32r, tag="qslot")
            for m in range(KD):
                if m == 0:
                    wt = wt0
                else:
                    wt = wqk_pool.tile([P, KD, P], f32r, tag="wqk")
                    nc.sync.dma_start(out=wt,
                                      in_=wq[m].rearrange("(k p) c -> p k c", p=P))
                ps = psum_mm.tile([P, CH], f32, tag="ps")
                for k in range(KD):
                    nc.tensor.matmul(ps, lhsT=wt[:, k, :],
                                     rhs=xw_sb[:, k, HALO:],
                                     start=(k == 0), stop=(k == KD - 1))
                if flags["use_bq"]:
                    nc.scalar.activation(qT[:, m, :], ps,
                                         mybir.ActivationFunctionType.Identity,
                                         bias=bias_ap(I_BQ, m))
                else:
                    nc.scalar.copy(qT[:, m, :], ps)
            # kT [d, win] with elu(x)+1 = relu(x) + exp(min(x, 0))
            kT = ap_.tile([P, KD, WIN], f32r)
            wvts = []
            for m in range(KD):
                if m == KD - 1:
                    # first v-weight half streams in behind the wk tiles so
                    # the v-proj can start right as kT finishes
                    wvt0 = wv_pool.tile([P, KD, 512], f32r, tag="wv")
                    wvts.append(wvt0)
                    nc.sync.dma_start(
                        out=wvt0,
                        in_=wv.rearrange("(k p) n -> p k n", p=P)[:, :, :512])
                wt = wqk_pool.tile([P, KD, P], f32r, tag="wqk")
                nc.sync.dma_start(out=wt,
                                  in_=wk[m].rearrange("(k p) c -> p k c", p=P))
                for half in range(2):
                    sl = slice(320 * half, 320 * (half + 1))
                    ps = psum_mm.tile([P, 320], f32, tag="ps")
                    for k in range(KD):
                        nc.tensor.matmul(ps, lhsT=wt[:, k, :],
                                         rhs=xw_sb[:, k, sl],
                                         start=(k == 0), stop=(k == KD - 1))
                    bk_b = bias_ap(I_BK, m) if flags["use_bk"] else zero_b
                    a = lnt.tile([P, 320], f32, tag="elu")
                    nc.scalar.activation(a, ps,
                                         mybir.ActivationFunctionType.Relu,
                                         bias=bk_b)
                    mn = lnt.tile([P, 320], f32, tag="elu")
                    nc.vector.tensor_sub(mn, ps, a)
                    e = lnt.tile([P, 320], f32, tag="elu")
                    nc.scalar.activation(e, mn,
                                         mybir.ActivationFunctionType.Exp,
                                         bias=bk_b)
                    nc.vector.tensor_add(kT[:, m, sl], a, e)

            # v [win_tok, d] token-major
            vt = ap_.tile([P, MW, D], f32r)
            wvt1 = wv_pool.tile([P, KD, 512], f32r, tag="wv")
            wvts.append(wvt1)
            nc.sync.dma_start(
                out=wvt1,
                in_=wv.rearrange("(k p) n -> p k n", p=P)[:, :, 512:])
            mask_sb = ap_.tile([P, MW, CH], f32)
            nc.sync.dma_start(out=mask_sb,
                              in_=maskT.rearrange("(m p) q -> p m q", p=P))
            nc.sync.dma_start(out=sm, in_=smalls)
            nc.sync.dma_start(out=b1s, in_=b1v)
            for half in range(2):
                wvt = wvts[half]
                for m in range(MW):
                    ps = psum_mm.tile([P, CH], f32, tag="ps")
                    for k in range(KD):
                        nc.tensor.matmul(ps, lhsT=xw_sb[:, k, P * m:P * (m + 1)],
                                         rhs=wvt[:, k, :],
                                         start=(k == 0), stop=(k == KD - 1))
                    dst = vt[:, m, 512 * half:512 * (half + 1)]
                    if flags["use_bv"]:
                        nc.vector.tensor_add(dst, ps,
                                             bv_bc[:, 512 * half:512 * (half + 1)])
                    else:
                        nc.scalar.copy(dst, ps)

            # scoresT [win_tok, q] * maskT
            scoresM = ap_.tile([P, MW, CH], f32r)
            for m in range(MW):
                ps = psum_mm.tile([P, CH], f32, tag="ps")
                for k in range(KD):
                    nc.tensor.matmul(ps, lhsT=kT[:, k, P * m:P * (m + 1)],
                                     rhs=qT[:, k, :],
                                     start=(k == 0), stop=(k == KD - 1))
                nc.vector.tensor_mul(scoresM[:, m, :], ps, mask_sb[:, m, :])

            # attn_outT [d, q]  (reuses qT slot)
            at = ap_.tile([P, KD, CH], f32r, tag="qslot")
            for m in range(KD):
                ps = psum_mm.tile([P, CH], f32, tag="ps")
                for k in range(MW):
                    nc.tensor.matmul(ps, lhsT=vt[:, k, P * m:P * (m + 1)],
                                     rhs=scoresM[:, k, :],
                                     start=(k == 0), stop=(k == MW - 1))
                nc.scalar.copy(at[:, m, :], ps)

            # fastT + residual -> h, split in token halves; LN(A) overlaps
            # the B-half o-proj matmuls.
            wts = {}
            oseq = _halves_seq(KD, 2)
            for i, (t2, m) in enumerate(oseq):
                sl = tsl(t2)
                if t2 == 0:
                    wt = wqk_pool.tile([P, KD, P], f32r, tag="wqk")
                    nc.sync.dma_start(out=wt,
                                      in_=wo[m].rearrange("(k p) c -> p k c", p=P))
                    wts[m] = wt
                wt = wts[m]
                ps = psum_mm.tile([P, TT], f32, tag="ps")
                for k in range(KD):
                    nc.tensor.matmul(ps, lhsT=wt[:, k, :], rhs=at[:, k, sl],
                                     start=(k == 0), stop=(k == KD - 1))
                nc.vector.tensor_add(h[:, m, sl], ps, xw_sb[:, m, HALO + TT * t2:
                                                             HALO + TT * (t2 + 1)])
                if flags["use_bo"]:
                    nc.vector.tensor_scalar_add(h[:, m, sl], c(h[:, m, sl]),
                                                bias_ap(I_BO, m))
                if t2 == 0 and m == KD - 1:
                    layernorm_half(I_GF, I_BF, 0, flags["use_gbf"])
            layernorm_half(I_GF, I_BF, 1, flags["use_gbf"])

        # ---------------- MLP stack (fp8 DoubleRow + correction) ----------
        UP_SHIFT = 28
        DN_SHIFT = 4
        with tc.tile_pool(name="w1p", bufs=10) as w1_pool, \
             tc.tile_pool(name="w2p", bufs=6) as w2_pool, \
             tc.tile_pool(name="ubp", bufs=6) as ub_pool, \
             tc.tile_pool(name="up", bufs=1) as u_pool:
            preload = {}
            for l in range(L):
                uc = u_pool.tile([P, 2, KU, CH], fp8, tag="uc")
                # ---- up-proj (A leads B by UP_SHIFT m-chunks)
                wts = preload
                preload = {}
                for i, (t2, m) in enumerate(_halves_seq(KU, UP_SHIFT)):
                    sl = tsl(t2)
                    g, j = divmod(m, 4)
                    if t2 == 0 and j == 0 and g not in wts:
                        w4 = w1_pool.tile([P, 4, 2, KD, P], fp8, tag="w1t")
                        nc.sync.dma_start(out=w4, in_=w1c[l, g])
                        wts[g] = w4
                    wt = wts[g]
                    ps = psum_mm.tile([P, TT], f32, tag="ps")
                    for kk in range(KD // 2):
                        nc.tensor.matmul(ps,
                                         lhsT=wt[:, j, 1, 2 * kk:2 * kk + 2, :],
                                         rhs=hc[:, 0, 2 * kk:2 * kk + 2, sl],
                                         start=(kk == 0), stop=False,
                                         perf_mode=DRM)
                    for k in range(KD):
                        nc.tensor.matmul(ps, lhsT=wt[:, j, :, k, :],
                                         rhs=hc[:, :, k, sl],
                                         start=False, stop=(k == KD - 1),
                                         perf_mode=DRM)
                    ub = ub_pool.tile([P, TT], bf16, tag="ub")
                    nc.scalar.activation(ub, ps,
                                         mybir.ActivationFunctionType.Gelu,
                                         bias=(b1s[:, l, m:m + 1]
                                               if flags["use_b1"] else zero_b),
                                         scale=inv_sw)
                    fp8_pair(uc[:, 0, m, sl], uc[:, 1, m, sl], ub)
                # ---- down-proj (A leads B by DN_SHIFT; LN(A) after last A)
                # prefetch the next layer's first up-proj weight groups now:
                # the SP DMA queue is clear here, so these transfers run
                # during the down-pass instead of bunching at the boundary.
                if l + 1 < L:
                    for g in range(2):
                        w4 = w1_pool.tile([P, 4, 2, KD, P], fp8, tag="w1t")
                        nc.sync.dma_start(out=w4, in_=w1c[l + 1, g])
                        preload[g] = w4
                wts = {}
                dseq = _halves_seq(KD, DN_SHIFT)
                for i, (t2, m) in enumerate(dseq):
                    sl = tsl(t2)
                    if t2 == 0:
                        wt = w2_pool.tile([P, 2, KU, P], fp8, tag="w2t")
                        nc.sync.dma_start(out=wt, in_=w2c[l, m])
                        wts[m] = wt
                    wt = wts[m]
                    ps = psum_mm.tile([P, TT], f32, tag="ps")
                    for kk in range(KU // 2):
                        nc.tensor.matmul(ps, lhsT=wt[:, 1, 2 * kk:2 * kk + 2, :],
                                         rhs=uc[:, 0, 2 * kk:2 * kk + 2, sl],
                                         start=(kk == 0), stop=False,
                                         perf_mode=DRM)
                    for k in range(KU):
                        nc.tensor.matmul(ps, lhsT=wt[:, :, k, :],
                                         rhs=uc[:, :, k, sl],
                                         start=False, stop=(k == KU - 1),
                                         perf_mode=DRM)
                    with nc.allow_low_precision(reason="f32r=fp32 bits"):
                        nc.vector.scalar_tensor_tensor(
                            h[:, m, sl], ps, 1.0 / SW, c(h[:, m, sl]),
                            mybir.AluOpType.mult, mybir.AluOpType.add)
                    if flags["use_b2"]:
                        nc.vector.tensor_scalar_add(h[:, m, sl], c(h[:, m, sl]),
                                                    bias_ap(I_B2(l), m))
                    if t2 == 0 and m == KD - 1:
                        layernorm_half(I_GC(l), I_BE(l), 0, flags["use_gbc"])
                layernorm_half(I_GC(l), I_BE(l), 1, flags["use_gbc"])

        # ---------------- LM head (fp8 DoubleRow + correction) ------------
        with tc.tile_pool(name="whp", bufs=10) as wh_pool, \
             tc.tile_pool(name="outp", bufs=12) as out_pool, \
             tc.tile_pool(name="bhp", bufs=2) as bh_pool:
            for n in range(NVC):
                wht = wh_pool.tile([P, 2, KD, NV], fp8)
                nc.sync.dma_start(out=wht, in_=whc[n])

                if flags["use_bh"]:
                    bh_bc = bh_pool.tile([P, NV], f32)
                    src = bass.AP(tensor=bhv.tensor,
                                  offset=bhv.offset + NV * n * 4,
                                  ap=[[0, P], [4, NV]])
                    nc.sync.dma_start(out=bh_bc, in_=src)
                for m in range(4):
                    ps = psum_mm.tile([P, NV], f32, tag="ps")
                    for kk in range(KD // 2):
                        nc.tensor.matmul(
                            ps,
                            lhsT=hc[:, 0, 2 * kk:2 * kk + 2, P * m:P * (m + 1)],
                            rhs=wht[:, 1, 2 * kk:2 * kk + 2, :],
                            start=(kk == 0), stop=False, perf_mode=DRM)
                    for k in range(KD):
                        nc.tensor.matmul(
                            ps,
                            lhsT=hc[:, :, k, P * m:P * (m + 1)],
                            rhs=wht[:, :, k, :],
                            start=False, stop=(k == KD - 1), perf_mode=DRM)
                    ot = out_pool.tile([P, NV], bf16)
                    if flags["use_bh"]:
                        sc = out_pool.tile([P, NV], f32, tag="osc")
                        nc.scalar.activation(sc, ps,
                                             mybir.ActivationFunctionType.Identity,
                                             bias=zero_b, scale=inv_sw)
                        nc.vector.tensor_add(ot, sc, bh_bc)
                    else:
                        nc.scalar.activation(ot, ps,
                                             mybir.ActivationFunctionType.Identity,
                                             bias=zero_b, scale=inv_sw)
                    nc.sync.dma_start(out=out[P * m:P * (m + 1), NV * n:NV * (n + 1)],
                                      in_=ot)

    nc.compile()
    return nc


def _get_program(flags):
    key = tuple(sorted(flags.items()))
    if key not in _CACHE:
        _CACHE[key] = _build_program(flags)
    return _CACHE[key]


def _f8pair(w):
    """w (f32) -> (d8, w8) fp8 e4m3 blocks of SW*w: SW*w ~ w8 + d8."""
    ws = np.clip(w * SW, -240.0, 240.0)
    w8 = ws.astype(E4NP)
    d8 = (ws - w8.astype(np.float32)).astype(E4NP)
    return d8, w8


def kernel(x, Wq, bq, Wk, bk, Wv, bv, Wo, bo, decay_param, g_fast, b_fast,
           W1, b1, W2, b2, g_cms, beta_cms, Wh, bh):
    x = np.asarray(x, np.float32)
    Wq, Wk, Wv, Wo = (np.asarray(a, np.float32) for a in (Wq, Wk, Wv, Wo))
    bq, bk, bv, bo = (np.asarray(a, np.float32) for a in (bq, bk, bv, bo))
    g_fast, b_fast = np.asarray(g_fast, np.float32), np.asarray(b_fast, np.float32)
    W1, W2 = np.asarray(W1, np.float32), np.asarray(W2, np.float32)
    b1, b2 = np.asarray(b1, np.float32), np.asarray(b2, np.float32)
    g_cms, beta_cms = np.asarray(g_cms, np.float32), np.asarray(beta_cms, np.float32)
    Wh, bh = np.asarray(Wh, np.float32), np.asarray(bh, np.float32)
    decay = float(1.0 / (1.0 + np.exp(-np.float64(np.asarray(decay_param)))))
    if decay ** HALO > 1e-12:
        raise NotImplementedError(
            f"decay={decay} too close to 1 for banded attention (halo={HALO})")

    flags = {
        "use_bq": bool(np.any(bq)), "use_bk": bool(np.any(bk)),
        "use_bv": bool(np.any(bv)), "use_bo": bool(np.any(bo)),
        "use_b1": bool(np.any(b1)), "use_b2": bool(np.any(b2)),
        "use_bh": bool(np.any(bh)),
        "use_gbf": bool(np.any(g_fast != 1.0) or np.any(b_fast)),
        "use_gbc": bool(np.any(g_cms != 1.0) or np.any(beta_cms)),
    }
    nc = _get_program(flags)

    # host-side weight layout prep (shared by all cores)
    wq_h = np.ascontiguousarray(Wq.reshape(D, KD, P).transpose(1, 0, 2))
    wk_h = np.ascontiguousarray(Wk.reshape(D, KD, P).transpose(1, 0, 2))
    wo_h = np.ascontiguousarray(Wo.reshape(D, KD, P).transpose(1, 0, 2))

    # fp8 pairs, SBUF-layout (partition-major): [l, m, p, b, k, c]
    d1, w1_8 = _f8pair(W1)                      # [L, D, ED]
    a = w1_8.reshape(L, KD, P, KU, P).transpose(0, 3, 2, 1, 4)
    d = d1.reshape(L, KD, P, KU, P).transpose(0, 3, 2, 1, 4)
    w1c_h = np.stack([d, a], axis=3)                            # [L,KU,P,2,KD,P]
    w1c_h = np.ascontiguousarray(
        w1c_h.reshape(L, KU // 4, 4, P, 2, KD, P).transpose(0, 1, 3, 2, 4, 5, 6))
    d2, w2_8 = _f8pair(W2)                      # [L, ED, D]
    a = w2_8.reshape(L, KU, P, KD, P).transpose(0, 3, 2, 1, 4)
    d = d2.reshape(L, KU, P, KD, P).transpose(0, 3, 2, 1, 4)
    w2c_h = np.ascontiguousarray(np.stack([d, a], axis=3))      # [L,KD,P,2,KU,P]
    dh_, wh_8 = _f8pair(Wh)                     # [D, V]
    a = wh_8.reshape(KD, P, NVC, NV).transpose(2, 1, 0, 3)      # [n, p, k, nv]
    d = dh_.reshape(KD, P, NVC, NV).transpose(2, 1, 0, 3)
    whc_h = np.ascontiguousarray(np.stack([d, a], axis=2))      # [NVC,P,2,KD,NV]

    smalls = np.zeros((NS, D), np.float32)
    smalls[I_BQ], smalls[I_BK], smalls[I_BV], smalls[I_BO] = bq, bk, bv, bo
    smalls[I_GF], smalls[I_BF] = g_fast, b_fast
    for l in range(L):
        smalls[I_B2(l)], smalls[I_GC(l)], smalls[I_BE(l)] = b2[l], g_cms[l], beta_cms[l]
    smalls = np.ascontiguousarray(
        smalls.reshape(NS, KD, P).transpose(2, 0, 1))        # [P, NS, KD]
    b1_r = np.ascontiguousarray(
        b1.reshape(L, KU, P).transpose(2, 0, 1))             # [P, L, KU]

    shared = {"wq": wq_h, "wk": wk_h, "wo": wo_h, "wv": Wv,
              "w1c": w1c_h, "w2c": w2c_h, "whc": whc_h,
              "smalls": smalls, "b1v": b1_r,
              "onesc": np.ones((P, 1), np.float32),
              "onesr": np.ones((1, P), np.float32)}
    if flags["use_bh"]:
        shared["bhv"] = bh
    if flags["use_bv"]:
        shared["bvv"] = bv

    in_maps = []
    i_loc = np.arange(CH)[:, None]
    j_loc = np.arange(WIN)[None, :]
    for c in range(NCORES):
        bidx, start = divmod(c * CH, S)
        w0 = start - HALO
        xwin = np.zeros((WIN, D), np.float32)
        lo = max(w0, 0)
        xwin[lo - w0:, :] = x[bidx, lo:start + CH]
        xw_h = np.ascontiguousarray(xwin.T)  # [D, WIN]
        expo = (HALO + i_loc) - j_loc - 1
        valid = (expo >= 0) & (w0 + j_loc >= 0)
        maskq = np.where(valid,
                         np.power(np.float64(decay), np.maximum(expo, 0)),
                         0.0).astype(np.float32)          # [CH, WIN]
        mask_h = np.ascontiguousarray(maskq.T)            # [WIN, CH]
        in_maps.append({**shared, "xw": xw_h, "maskT": mask_h})

    res = run_bass_kernel_spmd(nc, in_maps, list(range(NCORES)), trace=TRACE)
    outs = [res.results[c]["out"].astype(np.float32) for c in range(NCORES)]
    full = np.concatenate(outs, axis=0).reshape(B, S, V)
    kernel.last_result = res
    return full


# revision 57
# speedup vs baseline: 1.0231x; 1.0081x over previous
"""Trainium2 Bass kernel for nn_HOPE_7275674599449.

Decay-masked fast-weight attention + 4-layer MLP stack + LM head,
data-parallel over 8 NeuronCores (512 tokens each, 128-token halo for
the decay-banded attention; decay^128 underflows fp32 so the banding
is numerically exact).

Per-core program (feature-major activations [d_partitions, tokens]):
  - q/k/v/o projections + scores + attn in f32r (fp22 on PE, 1 cyc/row)
  - MLP + LM head matmuls in fp8e4 DoubleRow (0.5 cyc/row, 2 k-tiles
    per instruction) with an error-compensated split:
        W@h = W8@h8 + (dW8@h8 + W8@dh8),  W8 = fp8(128*W), dW8 = fp8(128*W - W8)
    The two correction products share one DoubleRow instruction per
    k-tile, so the whole thing costs 12 slot-pairs per 8 k-tiles =
    0.75x bf16 while landing ~bf16 accuracy (measured rel ~3e-3).
  - LayerNorms: partition-dim reductions via ones-matmul on PE (f32r),
    per-token stats broadcast via ones-matmul, elementwise on DVE/ACT.
  - Token-half (A/B) software pipeline: the o-proj / MLP matmuls and
    each LayerNorm are split into 256-token halves and emitted in a
    shifted order, so the LN + fp8-conversion chain of one half runs
    on DVE/ACT/Pool while the PE crunches the other half.
"""

import sys

sys.path.insert(0, "/opt/trn_rl_repo")

from contextlib import ExitStack

import ml_dtypes
import numpy as np

import concourse.bass as bass
import concourse.tile as tile
from concourse import bacc, mybir
from concourse.bass_utils import run_bass_kernel_spmd

P = 128
B, S, D, L, V = 2, 2048, 1024, 4, 32000
ED = 4 * D              # MLP hidden
CH = 512                # tokens per core
TT = CH // 2            # token half
HALO = 128
WIN = HALO + CH         # 640
KD = D // P             # 8
KU = ED // P            # 32
MW = WIN // P           # 5 window token chunks
NV = 500                # head free-dim chunk
NVC = V // NV           # 64
NCORES = 8
EPS = 1e-5
SW = 128.0              # fp8 weight pre-scale (power of 2, exact)

f32 = mybir.dt.float32
f32r = mybir.dt.float32r
bf16 = mybir.dt.bfloat16
fp8 = mybir.dt.float8e4
DRM = mybir.MatmulPerfMode.DoubleRow
E4NP = ml_dtypes.float8_e4m3   # TRN e4m3 (max 240)

# smalls stacking indices (rows of the [18, D] f32 "smalls" tensor)
I_BQ, I_BK, I_BV, I_BO, I_GF, I_BF = 0, 1, 2, 3, 4, 5
def I_B2(l): return 6 + 3 * l
def I_GC(l): return 7 + 3 * l
def I_BE(l): return 8 + 3 * l
NS = 6 + 3 * L

TRACE = False          # set by test.py for profiled runs
_CACHE = {}


def _halves_seq(n, shift):
    """Emission order (half, m): A leads B by `shift` m-chunks."""
    seq = [(0, m) for m in range(min(shift, n))]
    for m in range(shift, n):
        seq.append((1, m - shift))
        seq.append((0, m))
    for m in range(max(0, n - shift), n):
        seq.append((1, m))
    return seq


def _build_program(flags):
    """Build the per-core Bass/Tile program. flags: dict of use_* booleans."""
    nc = bacc.Bacc("TRN2", target_bir_lowering=False, debug=False,
                   num_devices=NCORES)

    xw = nc.dram_tensor("xw", [D, WIN], f32r, kind="ExternalInput").ap()
    maskT = nc.dram_tensor("maskT", [WIN, CH], f32, kind="ExternalInput").ap()
    wq = nc.dram_tensor("wq", [KD, D, P], f32r, kind="ExternalInput").ap()
    wk = nc.dram_tensor("wk", [KD, D, P], f32r, kind="ExternalInput").ap()
    wo = nc.dram_tensor("wo", [KD, D, P], f32r, kind="ExternalInput").ap()
    wv = nc.dram_tensor("wv", [D, D], f32r, kind="ExternalInput").ap()
    onesc = nc.dram_tensor("onesc", [P, 1], f32r, kind="ExternalInput").ap()
    onesr = nc.dram_tensor("onesr", [1, P], f32r, kind="ExternalInput").ap()
    # fp8 weight pairs: b=0 -> d (fp8 of scaled residual), b=1 -> hi fp8
    w1c = nc.dram_tensor("w1c", [L, KU // 4, P, 4, 2, KD, P], fp8,
                         kind="ExternalInput").ap()
    w2c = nc.dram_tensor("w2c", [L, KD, P, 2, KU, P], fp8, kind="ExternalInput").ap()
    whc = nc.dram_tensor("whc", [NVC, P, 2, KD, NV], fp8, kind="ExternalInput").ap()
    smalls = nc.dram_tensor("smalls", [P, NS, KD], f32, kind="ExternalInput").ap()
    b1v = nc.dram_tensor("b1v", [P, L, KU], f32, kind="ExternalInput").ap()
    bhv = None
    if flags["use_bh"]:
        bhv = nc.dram_tensor("bhv", [V], f32, kind="ExternalInput").ap()
    bvv = None
    if flags["use_bv"]:
        bvv = nc.dram_tensor("bvv", [D], f32, kind="ExternalInput").ap()
    out = nc.dram_tensor("out", [CH, V], bf16, kind="ExternalOutput").ap()

    # f32r tiles: PE reads fp32 bits, truncates to fp22, 1 cyc/row (vs 4
    # for fp32) when the moving free dim is >=256. The BIR verifier wants
    # every producer of an f32r-matmul operand typed f32r, so the tiles are
    # declared f32r and elementwise engines read them via .bitcast(f32).
    def c(ap): return ap.bitcast(f32)

    def tsl(t2):
        return slice(TT * t2, TT * (t2 + 1))

    with tile.TileContext(nc) as tc, ExitStack() as ctx:
        persist = ctx.enter_context(tc.tile_pool(name="persist", bufs=1))
        sqp = ctx.enter_context(tc.tile_pool(name="sqp", bufs=6))
        lnt = ctx.enter_context(tc.tile_pool(name="lnt", bufs=4))
        psum_mm = ctx.enter_context(
            tc.tile_pool(name="psum_mm", bufs=5, space="PSUM"))
        psum_s = ctx.enter_context(
            tc.tile_pool(name="psum_s", bufs=2, space="PSUM"))
        psum_bc = ctx.enter_context(
            tc.tile_pool(name="psum_bc", bufs=1, space="PSUM"))

        h = persist.tile([P, KD, CH], f32r)
        # hc: fp8 pair of h. [:,0,k,:] = h8, [:,1,k,:] = dh8 = fp8(h - h8)
        hc = persist.tile([P, 2, KD, CH], fp8)
        sm = persist.tile([P, NS, KD], f32)
        b1s = persist.tile([P, L, KU], f32)
        ones_col = persist.tile([P, 1], f32r)
        ones_row = persist.tile([1, P], f32r)
        eps_t = persist.tile([1, 1], f32)
        nc.vector.memset(eps_t, EPS)
        zero_b = persist.tile([P, 1], f32)
        nc.vector.memset(zero_b, 0.0)
        inv_sw = persist.tile([P, 1], f32)
        nc.vector.memset(inv_sw, 1.0 / SW)


        def bias_ap(idx, k):
            return sm[:, idx, k:k + 1]

        def fp8_pair(dst8, dstd, src):
            """dst8 = fp8(src); dstd = fp8(src - dst8). DVE does the sub."""
            with nc.allow_low_precision(reason="fp8 pair for DoubleRow"):
                nc.gpsimd.tensor_copy(dst8, src)
                nc.vector.tensor_sub(dstd, src, dst8)

        def fp8_pair_pool(dst8, dstd, src):
            """Same, entirely on Pool: keeps the LN-critical DVE free and
            avoids a cross-engine sem hop between copy and sub."""
            with nc.allow_low_precision(reason="fp8 pair for DoubleRow"):
                nc.gpsimd.tensor_copy(dst8, src)
                nc.gpsimd.tensor_sub(dstd, src, dst8)

        def layernorm_half(g_idx, b_idx, t2, apply_gb):
            """h[:, :, half] = LN(h)*g + b over d; refresh hc half."""
            sl = tsl(t2)
            ps_s = psum_s.tile([1, TT], f32, tag="lnsum")
            for k in range(KD):
                nc.tensor.matmul(ps_s, lhsT=ones_col, rhs=h[:, k, sl],
                                 start=(k == 0), stop=(k == KD - 1))
            ps_q = psum_s.tile([1, TT], f32, tag="lnsum")
            for k in range(KD):
                sq = sqp.tile([P, TT], f32r, tag="sq")
                nc.scalar.square(sq, c(h[:, k, sl]))
                nc.tensor.matmul(ps_q, lhsT=ones_col, rhs=sq,
                                 start=(k == 0), stop=(k == KD - 1))
            mean = lnt.tile([1, TT], f32r, tag="lnstat")
            nc.scalar.mul(mean, ps_s, 1.0 / D)
            ex2 = lnt.tile([1, TT], f32, tag="lnstat")
            nc.scalar.mul(ex2, ps_q, 1.0 / D)
            var = lnt.tile([1, TT], f32, tag="lnstat")
            nc.vector.tensor_mul(var, c(mean), c(mean))
            nc.vector.tensor_sub(var, ex2, var)
            std = lnt.tile([1, TT], f32, tag="lnstat")
            nc.scalar.activation(std, var,
                                 mybir.ActivationFunctionType.Sqrt, bias=eps_t)
            rstd = lnt.tile([1, TT], f32r, tag="lnstat")
            with nc.allow_low_precision(reason="f32r carries full fp32 bits"):
                nc.vector.reciprocal(rstd, std)
            bc2 = psum_bc.tile([P, 2, TT], f32, tag="bc")
            ps_mb, ps_rb = bc2[:, 0, :], bc2[:, 1, :]
            nc.tensor.matmul(ps_mb, lhsT=ones_row, rhs=mean,
                             start=True, stop=True)
            nc.tensor.matmul(ps_rb, lhsT=ones_row, rhs=rstd,
                             start=True, stop=True)
            for k in range(KD):
                t = lnt.tile([P, TT], f32, tag="lntmp")
                nc.vector.tensor_sub(t, c(h[:, k, sl]), ps_mb)
                if apply_gb:
                    nc.vector.tensor_mul(t, t, ps_rb)
                    nc.scalar.activation(h[:, k, sl], t,
                                         mybir.ActivationFunctionType.Identity,
                                         bias=bias_ap(b_idx, k),
                                         scale=bias_ap(g_idx, k))
                else:
                    # g==1, b==0: the normalized value IS h
                    with nc.allow_low_precision(reason="f32r=fp32 bits"):
                        nc.vector.tensor_mul(h[:, k, sl], t, ps_rb)
                fp8_pair_pool(hc[:, 0, k, sl], hc[:, 1, k, sl],
                              c(h[:, k, sl]))

        # ---------------- attention ----------------
        with tc.tile_pool(name="attn", bufs=1) as ap_, \
             tc.tile_pool(name="wqk", bufs=6) as wqk_pool, \
             tc.tile_pool(name="wvp", bufs=2) as wv_pool:
            xw_sb = ap_.tile([P, KD, WIN], f32r)
            wq_r = [None] * KD
            # first q-proj weight tile before everything else: it gates the
            # very first matmul
            wt0 = wqk_pool.tile([P, KD, P], f32r, tag="wqk")
            nc.sync.dma_start(out=wt0,
                              in_=wq[0].rearrange("(k p) c -> p k c", p=P))
            for k in range(KD):
                nc.sync.dma_start(
                    out=xw_sb[:, k, :],
                    in_=xw.rearrange("(k p) t -> p k t", p=P)[:, k, :])
            nc.sync.dma_start(out=ones_col, in_=onesc)
            nc.sync.dma_start(out=ones_row, in_=onesr)
            if flags["use_bv"]:
                bv_bc = ap_.tile([P, D], f32)
                src = bass.AP(tensor=bvv.tensor, offset=bvv.offset,
                              ap=[[0, P], bvv.ap[0]])
                nc.sync.dma_start(out=bv_bc, in_=src)

            # qT [d, q]
            qT = ap_.tile([P, KD, CH], f32r, tag="qslot")
            for m in range(KD):
                if m == 0:
                    wt = wt0
                else:
                    wt = wqk_pool.tile([P, KD, P], f32r, tag="wqk")
                    nc.sync.dma_start(out=wt,
                                      in_=wq[m].rearrange("(k p) c -> p k c", p=P))
                ps = psum_mm.tile([P, CH], f32, tag="ps")
                for k in range(KD):
                    nc.tensor.matmul(ps, lhsT=wt[:, k, :],
                                     rhs=xw_sb[:, k, HALO:],
                                     start=(k == 0), stop=(k == KD - 1))
                if flags["use_bq"]:
                    nc.scalar.activation(qT[:, m, :], ps,
                                         mybir.ActivationFunctionType.Identity,
                                         bias=bias_ap(I_BQ, m))
                else:
                    nc.scalar.copy(qT[:, m, :], ps)
            # kT [d, win] with elu(x)+1 = relu(x) + exp(min(x, 0))
            kT = ap_.tile([P, KD, WIN], f32r)
            wvts = []
            for m in range(KD):
                if m == KD - 1:
                    # first v-weight half streams in behind the wk tiles so
                    # the v-proj can start right as kT finishes
                    wvt0 = wv_pool.tile([P, KD, 512], f32r, tag="wv")
                    wvts.append(wvt0)
                    nc.sync.dma_start(
                        out=wvt0,
                        in_=wv.rearrange("(k p) n -> p k n", p=P)[:, :, :512])
                wt = wqk_pool.tile([P, KD, P], f32r, tag="wqk")
                nc.sync.dma_start(out=wt,
                                  in_=wk[m].rearrange("(k p) c -> p k c", p=P))
                for half in range(2):
                    sl = slice(320 * half, 320 * (half + 1))
                    ps = psum_mm.tile([P, 320], f32, tag="ps")
                    for k in range(KD):
                        nc.tensor.matmul(ps, lhsT=wt[:, k, :],
                                         rhs=xw_sb[:, k, sl],
                                         start=(k == 0), stop=(k == KD - 1))
                    bk_b = bias_ap(I_BK, m) if flags["use_bk"] else zero_b
                    a = lnt.tile([P, 320], f32, tag="elu")
                    nc.scalar.activation(a, ps,
                                         mybir.ActivationFunctionType.Relu,
                                         bias=bk_b)
                    mn = lnt.tile([P, 320], f32, tag="elu")
                    nc.vector.tensor_sub(mn, ps, a)
                    e = lnt.tile([P, 320], f32, tag="elu")
                    nc.scalar.activation(e, mn,
                                         mybir.ActivationFunctionType.Exp,
                                         bias=bk_b)
                    nc.vector.tensor_add(kT[:, m, sl], a, e)

            # v [win_tok, d] token-major
            vt = ap_.tile([P, MW, D], f32r)
            wvt1 = wv_pool.tile([P, KD, 512], f32r, tag="wv")
            wvts.append(wvt1)
            nc.sync.dma_start(
                out=wvt1,
                in_=wv.rearrange("(k p) n -> p k n", p=P)[:, :, 512:])
            mask_sb = ap_.tile([P, MW, CH], f32)
            nc.sync.dma_start(out=mask_sb,
                              in_=maskT.rearrange("(m p) q -> p m q", p=P))
            nc.sync.dma_start(out=sm, in_=smalls)
            nc.sync.dma_start(out=b1s, in_=b1v)
            for half in range(2):
                wvt = wvts[half]
                for m in range(MW):
                    ps = psum_mm.tile([P, CH], f32, tag="ps")
                    for k in range(KD):
                        nc.tensor.matmul(ps, lhsT=xw_sb[:, k, P * m:P * (m + 1)],
                                         rhs=wvt[:, k, :],
                                         start=(k == 0), stop=(k == KD - 1))
                    dst = vt[:, m, 512 * half:512 * (half + 1)]
                    if flags["use_bv"]:
                        nc.vector.tensor_add(dst, ps,
                                             bv_bc[:, 512 * half:512 * (half + 1)])
                    else:
                        nc.scalar.copy(dst, ps)

            # scoresT [win_tok, q] * maskT, banded: key-chunk m only
            # reaches queries in SBAND[m] (decay^128 underflows to exactly
            # 0 in fp32, so everything outside the band is zero anyway).
            SBAND = [(0, 256), (0, 256), (0, 512), (256, 512), (256, 512)]
            scoresM = ap_.tile([P, MW, CH], f32r)
            for m in range(MW):
                lo, hi = SBAND[m]
                ps = psum_mm.tile([P, CH], f32, tag="ps")
                for k in range(KD):
                    nc.tensor.matmul(ps[:, lo:hi],
                                     lhsT=kT[:, k, P * m:P * (m + 1)],
                                     rhs=qT[:, k, lo:hi],
                                     start=(k == 0), stop=(k == KD - 1))
                nc.vector.tensor_mul(scoresM[:, m, lo:hi], ps[:, lo:hi],
                                     mask_sb[:, m, lo:hi])

            # attn_outT [d, q] (reuses qT slot): query half A sees key
            # chunks 0-2, half B sees 2-4; the rest are exactly zero.
            AKR = [(0, 3), (2, 5)]
            at = ap_.tile([P, KD, CH], f32r, tag="qslot")
            for m in range(KD):
                for t2 in range(2):
                    lo = TT * t2
                    k0, k1 = AKR[t2]
                    ps = psum_mm.tile([P, TT], f32, tag="ps")
                    for k in range(k0, k1):
                        nc.tensor.matmul(ps, lhsT=vt[:, k, P * m:P * (m + 1)],
                                         rhs=scoresM[:, k, lo:lo + TT],
                                         start=(k == k0), stop=(k == k1 - 1))
                    nc.scalar.copy(at[:, m, lo:lo + TT], ps)

            # fastT + residual -> h, split in token halves; LN(A) overlaps
            # the B-half o-proj matmuls.
            wts = {}
            oseq = _halves_seq(KD, 2)
            for i, (t2, m) in enumerate(oseq):
                sl = tsl(t2)
                if t2 == 0:
                    wt = wqk_pool.tile([P, KD, P], f32r, tag="wqk")
                    nc.sync.dma_start(out=wt,
                                      in_=wo[m].rearrange("(k p) c -> p k c", p=P))
                    wts[m] = wt
                wt = wts[m]
                ps = psum_mm.tile([P, TT], f32, tag="ps")
                for k in range(KD):
                    nc.tensor.matmul(ps, lhsT=wt[:, k, :], rhs=at[:, k, sl],
                                     start=(k == 0), stop=(k == KD - 1))
                nc.vector.tensor_add(h[:, m, sl], ps, xw_sb[:, m, HALO + TT * t2:
                                                             HALO + TT * (t2 + 1)])
                if flags["use_bo"]:
                    nc.vector.tensor_scalar_add(h[:, m, sl], c(h[:, m, sl]),
                                                bias_ap(I_BO, m))
                if t2 == 0 and m == KD - 1:
                    layernorm_half(I_GF, I_BF, 0, flags["use_gbf"])
            layernorm_half(I_GF, I_BF, 1, flags["use_gbf"])

        # ---------------- MLP stack (fp8 DoubleRow + correction) ----------
        UP_SHIFT = 28
        DN_SHIFT = 4
        with tc.tile_pool(name="w1p", bufs=10) as w1_pool, \
             tc.tile_pool(name="w2p", bufs=6) as w2_pool, \
             tc.tile_pool(name="ubp", bufs=6) as ub_pool, \
             tc.tile_pool(name="up", bufs=1) as u_pool:
            preload = {}
            for l in range(L):
                uc = u_pool.tile([P, 2, KU, CH], fp8, tag="uc")
                # ---- up-proj (A leads B by UP_SHIFT m-chunks)
                wts = preload
                preload = {}
                for i, (t2, m) in enumerate(_halves_seq(KU, UP_SHIFT)):
                    sl = tsl(t2)
                    g, j = divmod(m, 4)
                    if t2 == 0 and j == 0 and g not in wts:
                        w4 = w1_pool.tile([P, 4, 2, KD, P], fp8, tag="w1t")
                        nc.sync.dma_start(out=w4, in_=w1c[l, g])
                        wts[g] = w4
                    wt = wts[g]
                    ps = psum_mm.tile([P, TT], f32, tag="ps")
                    for kk in range(KD // 2):
                        nc.tensor.matmul(ps,
                                         lhsT=wt[:, j, 1, 2 * kk:2 * kk + 2, :],
                                         rhs=hc[:, 0, 2 * kk:2 * kk + 2, sl],
                                         start=(kk == 0), stop=False,
                                         perf_mode=DRM)
                    for k in range(KD):
                        nc.tensor.matmul(ps, lhsT=wt[:, j, :, k, :],
                                         rhs=hc[:, :, k, sl],
                                         start=False, stop=(k == KD - 1),
                                         perf_mode=DRM)
                    ub = ub_pool.tile([P, TT], bf16, tag="ub")
                    nc.scalar.activation(ub, ps,
                                         mybir.ActivationFunctionType.Gelu,
                                         bias=(b1s[:, l, m:m + 1]
                                               if flags["use_b1"] else zero_b),
                                         scale=inv_sw)
                    fp8_pair(uc[:, 0, m, sl], uc[:, 1, m, sl], ub)
                # ---- down-proj (A leads B by DN_SHIFT; LN(A) after last A)
                # prefetch the next layer's first up-proj weight groups now:
                # the SP DMA queue is clear here, so these transfers run
                # during the down-pass instead of bunching at the boundary.
                if l + 1 < L:
                    for g in range(2):
                        w4 = w1_pool.tile([P, 4, 2, KD, P], fp8, tag="w1t")
                        nc.sync.dma_start(out=w4, in_=w1c[l + 1, g])
                        preload[g] = w4
                wts = {}
                dseq = _halves_seq(KD, DN_SHIFT)
                for i, (t2, m) in enumerate(dseq):
                    sl = tsl(t2)
                    if t2 == 0:
                        wt = w2_pool.tile([P, 2, KU, P], fp8, tag="w2t")
                        nc.sync.dma_start(out=wt, in_=w2c[l, m])
                        wts[m] = wt
                    wt = wts[m]
                    ps = psum_mm.tile([P, TT], f32, tag="ps")
                    for kk in range(KU // 2):
                        nc.tensor.matmul(ps, lhsT=wt[:, 1, 2 * kk:2 * kk + 2, :],
                                         rhs=uc[:, 0, 2 * kk:2 * kk + 2, sl],
                                         start=(kk == 0), stop=False,
                                         perf_mode=DRM)
                    for k in range(KU):
                        nc.tensor.matmul(ps, lhsT=wt[:, :, k, :],
                                         rhs=uc[:, :, k, sl],
                                         start=False, stop=(k == KU - 1),
                                         perf_mode=DRM)
                    with nc.allow_low_precision(reason="f32r=fp32 bits"):
                        nc.vector.scalar_tensor_tensor(
                            h[:, m, sl], ps, 1.0 / SW, c(h[:, m, sl]),
                            mybir.AluOpType.mult, mybir.AluOpType.add)
                    if flags["use_b2"]:
                        nc.vector.tensor_scalar_add(h[:, m, sl], c(h[:, m, sl]),
                                                    bias_ap(I_B2(l), m))
                    if t2 == 0 and m == KD - 1:
                        layernorm_half(I_GC(l), I_BE(l), 0, flags["use_gbc"])
                layernorm_half(I_GC(l), I_BE(l), 1, flags["use_gbc"])

        # ---------------- LM head (fp8 DoubleRow + correction) ------------
        with tc.tile_pool(name="whp", bufs=10) as wh_pool, \
             tc.tile_pool(name="outp", bufs=12) as out_pool, \
             tc.tile_pool(name="bhp", bufs=2) as bh_pool:
            for n in range(NVC):
                wht = wh_pool.tile([P, 2, KD, NV], fp8)
                nc.sync.dma_start(out=wht, in_=whc[n])

                if flags["use_bh"]:
                    bh_bc = bh_pool.tile([P, NV], f32)
                    src = bass.AP(tensor=bhv.tensor,
                                  offset=bhv.offset + NV * n * 4,
                                  ap=[[0, P], [4, NV]])
                    nc.sync.dma_start(out=bh_bc, in_=src)
                for m in range(4):
                    ps = psum_mm.tile([P, NV], f32, tag="ps")
                    for kk in range(KD // 2):
                        nc.tensor.matmul(
                            ps,
                            lhsT=hc[:, 0, 2 * kk:2 * kk + 2, P * m:P * (m + 1)],
                            rhs=wht[:, 1, 2 * kk:2 * kk + 2, :],
                            start=(kk == 0), stop=False, perf_mode=DRM)
                    for k in range(KD):
                        nc.tensor.matmul(
                            ps,
                            lhsT=hc[:, :, k, P * m:P * (m + 1)],
                            rhs=wht[:, :, k, :],
                            start=False, stop=(k == KD - 1), perf_mode=DRM)
                    ot = out_pool.tile([P, NV], bf16)
                    if flags["use_bh"]:
                        sc = out_pool.tile([P, NV], f32, tag="osc")
                        nc.scalar.activation(sc, ps,
                                             mybir.ActivationFunctionType.Identity,
                                             bias=zero_b, scale=inv_sw)
                        nc.vector.tensor_add(ot, sc, bh_bc)
                    else:
                        nc.scalar.activation(ot, ps,
                                             mybir.ActivationFunctionType.Identity,
                                             bias=zero_b, scale=inv_sw)
                    nc.sync.dma_start(out=out[P * m:P * (m + 1), NV * n:NV * (n + 1)],
                                      in_=ot)

    nc.compile()
    return nc


def _get_program(flags):
    key = tuple(sorted(flags.items()))
    if key not in _CACHE:
        _CACHE[key] = _build_program(flags)
    return _CACHE[key]


def _f8pair(w):
    """w (f32) -> (d8, w8) fp8 e4m3 blocks of SW*w: SW*w ~ w8 + d8."""
    ws = np.clip(w * SW, -240.0, 240.0)
    w8 = ws.astype(E4NP)
    d8 = (ws - w8.astype(np.float32)).astype(E4NP)
    return d8, w8


def kernel(x, Wq, bq, Wk, bk, Wv, bv, Wo, bo, decay_param, g_fast, b_fast,
           W1, b1, W2, b2, g_cms, beta_cms, Wh, bh):
    x = np.asarray(x, np.float32)
    Wq, Wk, Wv, Wo = (np.asarray(a, np.float32) for a in (Wq, Wk, Wv, Wo))
    bq, bk, bv, bo = (np.asarray(a, np.float32) for a in (bq, bk, bv, bo))
    g_fast, b_fast = np.asarray(g_fast, np.float32), np.asarray(b_fast, np.float32)
    W1, W2 = np.asarray(W1, np.float32), np.asarray(W2, np.float32)
    b1, b2 = np.asarray(b1, np.float32), np.asarray(b2, np.float32)
    g_cms, beta_cms = np.asarray(g_cms, np.float32), np.asarray(beta_cms, np.float32)
    Wh, bh = np.asarray(Wh, np.float32), np.asarray(bh, np.float32)
    decay = float(1.0 / (1.0 + np.exp(-np.float64(np.asarray(decay_param)))))
    if decay ** HALO > 1e-12:
        raise NotImplementedError(
            f"decay={decay} too close to 1 for banded attention (halo={HALO})")

    flags = {
        "use_bq": bool(np.any(bq)), "use_bk": bool(np.any(bk)),
        "use_bv": bool(np.any(bv)), "use_bo": bool(np.any(bo)),
        "use_b1": bool(np.any(b1)), "use_b2": bool(np.any(b2)),
        "use_bh": bool(np.any(bh)),
        "use_gbf": bool(np.any(g_fast != 1.0) or np.any(b_fast)),
        "use_gbc": bool(np.any(g_cms != 1.0) or np.any(beta_cms)),
    }
    nc = _get_program(flags)

    # host-side weight layout prep (shared by all cores)
    wq_h = np.ascontiguousarray(Wq.reshape(D, KD, P).transpose(1, 0, 2))
    wk_h = np.ascontiguousarray(Wk.reshape(D, KD, P).transpose(1, 0, 2))
    wo_h = np.ascontiguousarray(Wo.reshape(D, KD, P).transpose(1, 0, 2))

    # fp8 pairs, SBUF-layout (partition-major): [l, m, p, b, k, c]
    d1, w1_8 = _f8pair(W1)                      # [L, D, ED]
    a = w1_8.reshape(L, KD, P, KU, P).transpose(0, 3, 2, 1, 4)
    d = d1.reshape(L, KD, P, KU, P).transpose(0, 3, 2, 1, 4)
    w1c_h = np.stack([d, a], axis=3)                            # [L,KU,P,2,KD,P]
    w1c_h = np.ascontiguousarray(
        w1c_h.reshape(L, KU // 4, 4, P, 2, KD, P).transpose(0, 1, 3, 2, 4, 5, 6))
    d2, w2_8 = _f8pair(W2)                      # [L, ED, D]
    a = w2_8.reshape(L, KU, P, KD, P).transpose(0, 3, 2, 1, 4)
    d = d2.reshape(L, KU, P, KD, P).transpose(0, 3, 2, 1, 4)
    w2c_h = np.ascontiguousarray(np.stack([d, a], axis=3))      # [L,KD,P,2,KU,P]
    dh_, wh_8 = _f8pair(Wh)                     # [D, V]
    a = wh_8.reshape(KD, P, NVC, NV).transpose(2, 1, 0, 3)      # [n, p, k, nv]
    d = dh_.reshape(KD, P, NVC, NV).transpose(2, 1, 0, 3)
    whc_h = np.ascontiguousarray(np.stack([d, a], axis=2))      # [NVC,P,2,KD,NV]

    smalls = np.zeros((NS, D), np.float32)
    smalls[I_BQ], smalls[I_BK], smalls[I_BV], smalls[I_BO] = bq, bk, bv, bo
    smalls[I_GF], smalls[I_BF] = g_fast, b_fast
    for l in range(L):
        smalls[I_B2(l)], smalls[I_GC(l)], smalls[I_BE(l)] = b2[l], g_cms[l], beta_cms[l]
    smalls = np.ascontiguousarray(
        smalls.reshape(NS, KD, P).transpose(2, 0, 1))        # [P, NS, KD]
    b1_r = np.ascontiguousarray(
        b1.reshape(L, KU, P).transpose(2, 0, 1))             # [P, L, KU]

    shared = {"wq": wq_h, "wk": wk_h, "wo": wo_h, "wv": Wv,
              "w1c": w1c_h, "w2c": w2c_h, "whc": whc_h,
              "smalls": smalls, "b1v": b1_r,
              "onesc": np.ones((P, 1), np.float32),
              "onesr": np.ones((1, P), np.float32)}
    if flags["use_bh"]:
        shared["bhv"] = bh
    if flags["use_bv"]:
        shared["bvv"] = bv

    in_maps = []
    i_loc = np.arange(CH)[:, None]
    j_loc = np.arange(WIN)[None, :]
    for c in range(NCORES):
        bidx, start = divmod(c * CH, S)
        w0 = start - HALO
        xwin = np.zeros((WIN, D), np.float32)
        lo = max(w0, 0)
        xwin[lo - w0:, :] = x[bidx, lo:start + CH]
        xw_h = np.ascontiguousarray(xwin.T)  # [D, WIN]
        expo = (HALO + i_loc) - j_loc - 1
        valid = (expo >= 0) & (w0 + j_loc >= 0)
        maskq = np.where(valid,
                         np.power(np.float64(decay), np.maximum(expo, 0)),
                         0.0).astype(np.float32)          # [CH, WIN]
        mask_h = np.ascontiguousarray(maskq.T)            # [WIN, CH]
        in_maps.append({**shared, "xw": xw_h, "maskT": mask_h})

    res = run_bass_kernel_spmd(nc, in_maps, list(range(NCORES)), trace=TRACE)
    outs = [res.results[c]["out"].astype(np.float32) for c in range(NCORES)]
    full = np.concatenate(outs, axis=0).reshape(B, S, V)
    kernel.last_result = res
    return full


# revision 58
# speedup vs baseline: 1.0255x; 1.0024x over previous
"""Trainium2 Bass kernel for nn_HOPE_7275674599449.

Decay-masked fast-weight attention + 4-layer MLP stack + LM head,
data-parallel over 8 NeuronCores (512 tokens each, 128-token halo for
the decay-banded attention; decay^128 underflows fp32 so the banding
is numerically exact).

Per-core program (feature-major activations [d_partitions, tokens]):
  - q/k/v/o projections + scores + attn in f32r (fp22 on PE, 1 cyc/row)
  - MLP + LM head matmuls in fp8e4 DoubleRow (0.5 cyc/row, 2 k-tiles
    per instruction) with an error-compensated split:
        W@h = W8@h8 + (dW8@h8 + W8@dh8),  W8 = fp8(128*W), dW8 = fp8(128*W - W8)
    The two correction products share one DoubleRow instruction per
    k-tile, so the whole thing costs 12 slot-pairs per 8 k-tiles =
    0.75x bf16 while landing ~bf16 accuracy (measured rel ~3e-3).
  - LayerNorms: partition-dim reductions via ones-matmul on PE (f32r),
    per-token stats broadcast via ones-matmul, elementwise on DVE/ACT.
  - Token-half (A/B) software pipeline: the o-proj / MLP matmuls and
    each LayerNorm are split into 256-token halves and emitted in a
    shifted order, so the LN + fp8-conversion chain of one half runs
    on DVE/ACT/Pool while the PE crunches the other half.
"""

import sys

sys.path.insert(0, "/opt/trn_rl_repo")

from contextlib import ExitStack

import ml_dtypes
import numpy as np

import concourse.bass as bass
import concourse.tile as tile
from concourse import bacc, mybir
from concourse.bass_utils import run_bass_kernel_spmd

P = 128
B, S, D, L, V = 2, 2048, 1024, 4, 32000
ED = 4 * D              # MLP hidden
CH = 512                # tokens per core
TT = CH // 2            # token half
HALO = 128
WIN = HALO + CH         # 640
KD = D // P             # 8
KU = ED // P            # 32
MW = WIN // P           # 5 window token chunks
NV = 500                # head free-dim chunk
NVC = V // NV           # 64
NCORES = 8
EPS = 1e-5
SW = 128.0              # fp8 weight pre-scale (power of 2, exact)

f32 = mybir.dt.float32
f32r = mybir.dt.float32r
bf16 = mybir.dt.bfloat16
fp8 = mybir.dt.float8e4
DRM = mybir.MatmulPerfMode.DoubleRow
E4NP = ml_dtypes.float8_e4m3   # TRN e4m3 (max 240)

# smalls stacking indices (rows of the [18, D] f32 "smalls" tensor)
I_BQ, I_BK, I_BV, I_BO, I_GF, I_BF = 0, 1, 2, 3, 4, 5
def I_B2(l): return 6 + 3 * l
def I_GC(l): return 7 + 3 * l
def I_BE(l): return 8 + 3 * l
NS = 6 + 3 * L

TRACE = False          # set by test.py for profiled runs
_CACHE = {}


def _halves_seq(n, shift):
    """Emission order (half, m): A leads B by `shift` m-chunks."""
    seq = [(0, m) for m in range(min(shift, n))]
    for m in range(shift, n):
        seq.append((1, m - shift))
        seq.append((0, m))
    for m in range(max(0, n - shift), n):
        seq.append((1, m))
    return seq


def _build_program(flags):
    """Build the per-core Bass/Tile program. flags: dict of use_* booleans."""
    nc = bacc.Bacc("TRN2", target_bir_lowering=False, debug=False,
                   num_devices=NCORES)

    xw = nc.dram_tensor("xw", [D, WIN], f32r, kind="ExternalInput").ap()
    maskT = nc.dram_tensor("maskT", [WIN, CH], f32, kind="ExternalInput").ap()
    wq = nc.dram_tensor("wq", [KD, D, P], f32r, kind="ExternalInput").ap()
    wk = nc.dram_tensor("wk", [KD, D, P], f32r, kind="ExternalInput").ap()
    wo = nc.dram_tensor("wo", [KD, D, P], f32r, kind="ExternalInput").ap()
    wv = nc.dram_tensor("wv", [D, D], f32r, kind="ExternalInput").ap()
    onesc = nc.dram_tensor("onesc", [P, 1], f32r, kind="ExternalInput").ap()
    onesr = nc.dram_tensor("onesr", [1, P], f32r, kind="ExternalInput").ap()
    # fp8 weight pairs: b=0 -> d (fp8 of scaled residual), b=1 -> hi fp8
    w1c = nc.dram_tensor("w1c", [L, KU // 4, P, 4, 2, KD, P], fp8,
                         kind="ExternalInput").ap()
    w2c = nc.dram_tensor("w2c", [L, KD, P, 2, KU, P], fp8, kind="ExternalInput").ap()
    whc = nc.dram_tensor("whc", [NVC, P, 2, KD, NV], fp8, kind="ExternalInput").ap()
    smalls = nc.dram_tensor("smalls", [P, NS, KD], f32, kind="ExternalInput").ap()
    b1v = nc.dram_tensor("b1v", [P, L, KU], f32, kind="ExternalInput").ap()
    bhv = None
    if flags["use_bh"]:
        bhv = nc.dram_tensor("bhv", [V], f32, kind="ExternalInput").ap()
    bvv = None
    if flags["use_bv"]:
        bvv = nc.dram_tensor("bvv", [D], f32, kind="ExternalInput").ap()
    out = nc.dram_tensor("out", [CH, V], bf16, kind="ExternalOutput").ap()

    # f32r tiles: PE reads fp32 bits, truncates to fp22, 1 cyc/row (vs 4
    # for fp32) when the moving free dim is >=256. The BIR verifier wants
    # every producer of an f32r-matmul operand typed f32r, so the tiles are
    # declared f32r and elementwise engines read them via .bitcast(f32).
    def c(ap): return ap.bitcast(f32)

    def tsl(t2):
        return slice(TT * t2, TT * (t2 + 1))

    with tile.TileContext(nc) as tc, ExitStack() as ctx:
        persist = ctx.enter_context(tc.tile_pool(name="persist", bufs=1))
        sqp = ctx.enter_context(tc.tile_pool(name="sqp", bufs=6))
        lnt = ctx.enter_context(tc.tile_pool(name="lnt", bufs=4))
        psum_mm = ctx.enter_context(
            tc.tile_pool(name="psum_mm", bufs=6, space="PSUM"))
        psum_s = ctx.enter_context(
            tc.tile_pool(name="psum_s", bufs=1, space="PSUM"))
        psum_bc = ctx.enter_context(
            tc.tile_pool(name="psum_bc", bufs=1, space="PSUM"))

        h = persist.tile([P, KD, CH], f32r)
        # hc: fp8 pair of h. [:,0,k,:] = h8, [:,1,k,:] = dh8 = fp8(h - h8)
        hc = persist.tile([P, 2, KD, CH], fp8)
        sm = persist.tile([P, NS, KD], f32)
        b1s = persist.tile([P, L, KU], f32)
        ones_col = persist.tile([P, 1], f32r)
        ones_row = persist.tile([1, P], f32r)
        eps_t = persist.tile([1, 1], f32)
        nc.vector.memset(eps_t, EPS)
        zero_b = persist.tile([P, 1], f32)
        nc.vector.memset(zero_b, 0.0)
        inv_sw = persist.tile([P, 1], f32)
        nc.vector.memset(inv_sw, 1.0 / SW)


        def bias_ap(idx, k):
            return sm[:, idx, k:k + 1]

        def fp8_pair(dst8, dstd, src):
            """dst8 = fp8(src); dstd = fp8(src - dst8). DVE does the sub."""
            with nc.allow_low_precision(reason="fp8 pair for DoubleRow"):
                nc.gpsimd.tensor_copy(dst8, src)
                nc.vector.tensor_sub(dstd, src, dst8)

        def fp8_pair_pool(dst8, dstd, src):
            """Same, entirely on Pool: keeps the LN-critical DVE free and
            avoids a cross-engine sem hop between copy and sub."""
            with nc.allow_low_precision(reason="fp8 pair for DoubleRow"):
                nc.gpsimd.tensor_copy(dst8, src)
                nc.gpsimd.tensor_sub(dstd, src, dst8)

        def layernorm_half(g_idx, b_idx, t2, apply_gb):
            """h[:, :, half] = LN(h)*g + b over d; refresh hc half."""
            sl = tsl(t2)
            st2 = psum_s.tile([1, 2, TT], f32, tag="lnsum")
            ps_s, ps_q = st2[:, 0, :], st2[:, 1, :]
            for k in range(KD):
                nc.tensor.matmul(ps_s, lhsT=ones_col, rhs=h[:, k, sl],
                                 start=(k == 0), stop=(k == KD - 1))
            for k in range(KD):
                sq = sqp.tile([P, TT], f32r, tag="sq")
                nc.scalar.square(sq, c(h[:, k, sl]))
                nc.tensor.matmul(ps_q, lhsT=ones_col, rhs=sq,
                                 start=(k == 0), stop=(k == KD - 1))
            mean = lnt.tile([1, TT], f32r, tag="lnstat")
            nc.scalar.mul(mean, ps_s, 1.0 / D)
            ex2 = lnt.tile([1, TT], f32, tag="lnstat")
            nc.scalar.mul(ex2, ps_q, 1.0 / D)
            var = lnt.tile([1, TT], f32, tag="lnstat")
            nc.vector.tensor_mul(var, c(mean), c(mean))
            nc.vector.tensor_sub(var, ex2, var)
            std = lnt.tile([1, TT], f32, tag="lnstat")
            nc.scalar.activation(std, var,
                                 mybir.ActivationFunctionType.Sqrt, bias=eps_t)
            rstd = lnt.tile([1, TT], f32r, tag="lnstat")
            with nc.allow_low_precision(reason="f32r carries full fp32 bits"):
                nc.vector.reciprocal(rstd, std)
            bc2 = psum_bc.tile([P, 2, TT], f32, tag="bc")
            ps_mb, ps_rb = bc2[:, 0, :], bc2[:, 1, :]
            nc.tensor.matmul(ps_mb, lhsT=ones_row, rhs=mean,
                             start=True, stop=True)
            nc.tensor.matmul(ps_rb, lhsT=ones_row, rhs=rstd,
                             start=True, stop=True)
            for k in range(KD):
                t = lnt.tile([P, TT], f32, tag="lntmp")
                nc.vector.tensor_sub(t, c(h[:, k, sl]), ps_mb)
                if apply_gb:
                    nc.vector.tensor_mul(t, t, ps_rb)
                    nc.scalar.activation(h[:, k, sl], t,
                                         mybir.ActivationFunctionType.Identity,
                                         bias=bias_ap(b_idx, k),
                                         scale=bias_ap(g_idx, k))
                else:
                    # g==1, b==0: the normalized value IS h
                    with nc.allow_low_precision(reason="f32r=fp32 bits"):
                        nc.vector.tensor_mul(h[:, k, sl], t, ps_rb)
                fp8_pair_pool(hc[:, 0, k, sl], hc[:, 1, k, sl],
                              c(h[:, k, sl]))

        # ---------------- attention ----------------
        with tc.tile_pool(name="attn", bufs=1) as ap_, \
             tc.tile_pool(name="wqk", bufs=6) as wqk_pool, \
             tc.tile_pool(name="wvp", bufs=2) as wv_pool:
            xw_sb = ap_.tile([P, KD, WIN], f32r)
            wq_r = [None] * KD
            # first q-proj weight tile before everything else: it gates the
            # very first matmul
            wt0 = wqk_pool.tile([P, KD, P], f32r, tag="wqk")
            nc.sync.dma_start(out=wt0,
                              in_=wq[0].rearrange("(k p) c -> p k c", p=P))
            for k in range(KD):
                nc.sync.dma_start(
                    out=xw_sb[:, k, :],
                    in_=xw.rearrange("(k p) t -> p k t", p=P)[:, k, :])
            nc.sync.dma_start(out=ones_col, in_=onesc)
            nc.sync.dma_start(out=ones_row, in_=onesr)
            if flags["use_bv"]:
                bv_bc = ap_.tile([P, D], f32)
                src = bass.AP(tensor=bvv.tensor, offset=bvv.offset,
                              ap=[[0, P], bvv.ap[0]])
                nc.sync.dma_start(out=bv_bc, in_=src)

            # qT [d, q]
            qT = ap_.tile([P, KD, CH], f32r, tag="qslot")
            for m in range(KD):
                if m == 0:
                    wt = wt0
                else:
                    wt = wqk_pool.tile([P, KD, P], f32r, tag="wqk")
                    nc.sync.dma_start(out=wt,
                                      in_=wq[m].rearrange("(k p) c -> p k c", p=P))
                ps = psum_mm.tile([P, CH], f32, tag="ps")
                for k in range(KD):
                    nc.tensor.matmul(ps, lhsT=wt[:, k, :],
                                     rhs=xw_sb[:, k, HALO:],
                                     start=(k == 0), stop=(k == KD - 1))
                if flags["use_bq"]:
                    nc.scalar.activation(qT[:, m, :], ps,
                                         mybir.ActivationFunctionType.Identity,
                                         bias=bias_ap(I_BQ, m))
                else:
                    nc.scalar.copy(qT[:, m, :], ps)
            # kT [d, win] with elu(x)+1 = relu(x) + exp(min(x, 0))
            kT = ap_.tile([P, KD, WIN], f32r)
            wvts = []
            for m in range(KD):
                if m == KD - 1:
                    # first v-weight half streams in behind the wk tiles so
                    # the v-proj can start right as kT finishes
                    wvt0 = wv_pool.tile([P, KD, 512], f32r, tag="wv")
                    wvts.append(wvt0)
                    nc.sync.dma_start(
                        out=wvt0,
                        in_=wv.rearrange("(k p) n -> p k n", p=P)[:, :, :512])
                wt = wqk_pool.tile([P, KD, P], f32r, tag="wqk")
                nc.sync.dma_start(out=wt,
                                  in_=wk[m].rearrange("(k p) c -> p k c", p=P))
                for half in range(2):
                    sl = slice(320 * half, 320 * (half + 1))
                    ps = psum_mm.tile([P, 320], f32, tag="ps")
                    for k in range(KD):
                        nc.tensor.matmul(ps, lhsT=wt[:, k, :],
                                         rhs=xw_sb[:, k, sl],
                                         start=(k == 0), stop=(k == KD - 1))
                    bk_b = bias_ap(I_BK, m) if flags["use_bk"] else zero_b
                    a = lnt.tile([P, 320], f32, tag="elu")
                    nc.scalar.activation(a, ps,
                                         mybir.ActivationFunctionType.Relu,
                                         bias=bk_b)
                    mn = lnt.tile([P, 320], f32, tag="elu")
                    nc.vector.tensor_sub(mn, ps, a)
                    e = lnt.tile([P, 320], f32, tag="elu")
                    nc.scalar.activation(e, mn,
                                         mybir.ActivationFunctionType.Exp,
                                         bias=bk_b)
                    nc.vector.tensor_add(kT[:, m, sl], a, e)

            # v [win_tok, d] token-major
            vt = ap_.tile([P, MW, D], f32r)
            wvt1 = wv_pool.tile([P, KD, 512], f32r, tag="wv")
            wvts.append(wvt1)
            nc.sync.dma_start(
                out=wvt1,
                in_=wv.rearrange("(k p) n -> p k n", p=P)[:, :, 512:])
            mask_sb = ap_.tile([P, MW, CH], f32)
            nc.sync.dma_start(out=mask_sb,
                              in_=maskT.rearrange("(m p) q -> p m q", p=P))
            nc.sync.dma_start(out=sm, in_=smalls)
            nc.sync.dma_start(out=b1s, in_=b1v)
            for half in range(2):
                wvt = wvts[half]
                for m in range(MW):
                    ps = psum_mm.tile([P, CH], f32, tag="ps")
                    for k in range(KD):
                        nc.tensor.matmul(ps, lhsT=xw_sb[:, k, P * m:P * (m + 1)],
                                         rhs=wvt[:, k, :],
                                         start=(k == 0), stop=(k == KD - 1))
                    dst = vt[:, m, 512 * half:512 * (half + 1)]
                    if flags["use_bv"]:
                        nc.vector.tensor_add(dst, ps,
                                             bv_bc[:, 512 * half:512 * (half + 1)])
                    else:
                        nc.scalar.copy(dst, ps)

            # scoresT [win_tok, q] * maskT, banded: key-chunk m only
            # reaches queries in SBAND[m] (decay^128 underflows to exactly
            # 0 in fp32, so everything outside the band is zero anyway).
            SBAND = [(0, 256), (0, 256), (0, 512), (256, 512), (256, 512)]
            scoresM = ap_.tile([P, MW, CH], f32r)
            for m in range(MW):
                lo, hi = SBAND[m]
                ps = psum_mm.tile([P, CH], f32, tag="ps")
                for k in range(KD):
                    nc.tensor.matmul(ps[:, lo:hi],
                                     lhsT=kT[:, k, P * m:P * (m + 1)],
                                     rhs=qT[:, k, lo:hi],
                                     start=(k == 0), stop=(k == KD - 1))
                nc.vector.tensor_mul(scoresM[:, m, lo:hi], ps[:, lo:hi],
                                     mask_sb[:, m, lo:hi])

            # attn_outT [d, q] (reuses qT slot): query half A sees key
            # chunks 0-2, half B sees 2-4; the rest are exactly zero.
            AKR = [(0, 3), (2, 5)]
            at = ap_.tile([P, KD, CH], f32r, tag="qslot")
            for m in range(KD):
                for t2 in range(2):
                    lo = TT * t2
                    k0, k1 = AKR[t2]
                    ps = psum_mm.tile([P, TT], f32, tag="ps")
                    for k in range(k0, k1):
                        nc.tensor.matmul(ps, lhsT=vt[:, k, P * m:P * (m + 1)],
                                         rhs=scoresM[:, k, lo:lo + TT],
                                         start=(k == k0), stop=(k == k1 - 1))
                    nc.scalar.copy(at[:, m, lo:lo + TT], ps)

            # fastT + residual -> h, split in token halves; LN(A) overlaps
            # the B-half o-proj matmuls.
            wts = {}
            oseq = _halves_seq(KD, 2)
            for i, (t2, m) in enumerate(oseq):
                sl = tsl(t2)
                if t2 == 0:
                    wt = wqk_pool.tile([P, KD, P], f32r, tag="wqk")
                    nc.sync.dma_start(out=wt,
                                      in_=wo[m].rearrange("(k p) c -> p k c", p=P))
                    wts[m] = wt
                wt = wts[m]
                ps = psum_mm.tile([P, TT], f32, tag="ps")
                for k in range(KD):
                    nc.tensor.matmul(ps, lhsT=wt[:, k, :], rhs=at[:, k, sl],
                                     start=(k == 0), stop=(k == KD - 1))
                nc.vector.tensor_add(h[:, m, sl], ps, xw_sb[:, m, HALO + TT * t2:
                                                             HALO + TT * (t2 + 1)])
                if flags["use_bo"]:
                    nc.vector.tensor_scalar_add(h[:, m, sl], c(h[:, m, sl]),
                                                bias_ap(I_BO, m))
                if t2 == 0 and m == KD - 1:
                    layernorm_half(I_GF, I_BF, 0, flags["use_gbf"])
            layernorm_half(I_GF, I_BF, 1, flags["use_gbf"])

        # ---------------- MLP stack (fp8 DoubleRow + correction) ----------
        UP_SHIFT = 28
        DN_SHIFT = 4
        with tc.tile_pool(name="w1p", bufs=10) as w1_pool, \
             tc.tile_pool(name="w2p", bufs=6) as w2_pool, \
             tc.tile_pool(name="ubp", bufs=6) as ub_pool, \
             tc.tile_pool(name="up", bufs=1) as u_pool:
            preload = {}
            for l in range(L):
                uc = u_pool.tile([P, 2, KU, CH], fp8, tag="uc")
                # ---- up-proj (A leads B by UP_SHIFT m-chunks)
                wts = preload
                preload = {}
                for i, (t2, m) in enumerate(_halves_seq(KU, UP_SHIFT)):
                    sl = tsl(t2)
                    g, j = divmod(m, 4)
                    if t2 == 0 and j == 0 and g not in wts:
                        w4 = w1_pool.tile([P, 4, 2, KD, P], fp8, tag="w1t")
                        nc.sync.dma_start(out=w4, in_=w1c[l, g])
                        wts[g] = w4
                    wt = wts[g]
                    ps = psum_mm.tile([P, TT], f32, tag="ps")
                    for kk in range(KD // 2):
                        nc.tensor.matmul(ps,
                                         lhsT=wt[:, j, 1, 2 * kk:2 * kk + 2, :],
                                         rhs=hc[:, 0, 2 * kk:2 * kk + 2, sl],
                                         start=(kk == 0), stop=False,
                                         perf_mode=DRM)
                    for k in range(KD):
                        nc.tensor.matmul(ps, lhsT=wt[:, j, :, k, :],
                                         rhs=hc[:, :, k, sl],
                                         start=False, stop=(k == KD - 1),
                                         perf_mode=DRM)
                    ub = ub_pool.tile([P, TT], bf16, tag="ub")
                    nc.scalar.activation(ub, ps,
                                         mybir.ActivationFunctionType.Gelu,
                                         bias=(b1s[:, l, m:m + 1]
                                               if flags["use_b1"] else zero_b),
                                         scale=inv_sw)
                    fp8_pair(uc[:, 0, m, sl], uc[:, 1, m, sl], ub)
                # ---- down-proj (A leads B by DN_SHIFT; LN(A) after last A)
                # prefetch the next layer's first up-proj weight groups now:
                # the SP DMA queue is clear here, so these transfers run
                # during the down-pass instead of bunching at the boundary.
                if l + 1 < L:
                    for g in range(2):
                        w4 = w1_pool.tile([P, 4, 2, KD, P], fp8, tag="w1t")
                        nc.sync.dma_start(out=w4, in_=w1c[l + 1, g])
                        preload[g] = w4
                wts = {}
                dseq = _halves_seq(KD, DN_SHIFT)
                for i, (t2, m) in enumerate(dseq):
                    sl = tsl(t2)
                    if t2 == 0:
                        wt = w2_pool.tile([P, 2, KU, P], fp8, tag="w2t")
                        nc.sync.dma_start(out=wt, in_=w2c[l, m])
                        wts[m] = wt
                    wt = wts[m]
                    ps = psum_mm.tile([P, TT], f32, tag="ps")
                    for kk in range(KU // 2):
                        nc.tensor.matmul(ps, lhsT=wt[:, 1, 2 * kk:2 * kk + 2, :],
                                         rhs=uc[:, 0, 2 * kk:2 * kk + 2, sl],
                                         start=(kk == 0), stop=False,
                                         perf_mode=DRM)
                    for k in range(KU):
                        nc.tensor.matmul(ps, lhsT=wt[:, :, k, :],
                                         rhs=uc[:, :, k, sl],
                                         start=False, stop=(k == KU - 1),
                                         perf_mode=DRM)
                    with nc.allow_low_precision(reason="f32r=fp32 bits"):
                        nc.vector.scalar_tensor_tensor(
                            h[:, m, sl], ps, 1.0 / SW, c(h[:, m, sl]),
                            mybir.AluOpType.mult, mybir.AluOpType.add)
                    if flags["use_b2"]:
                        nc.vector.tensor_scalar_add(h[:, m, sl], c(h[:, m, sl]),
                                                    bias_ap(I_B2(l), m))
                    if t2 == 0 and m == KD - 1:
                        layernorm_half(I_GC(l), I_BE(l), 0, flags["use_gbc"])
                layernorm_half(I_GC(l), I_BE(l), 1, flags["use_gbc"])

        # ---------------- LM head (fp8 DoubleRow + correction) ------------
        with tc.tile_pool(name="whp", bufs=10) as wh_pool, \
             tc.tile_pool(name="outp", bufs=12) as out_pool, \
             tc.tile_pool(name="bhp", bufs=2) as bh_pool:
            for n in range(NVC):
                wht = wh_pool.tile([P, 2, KD, NV], fp8)
                nc.sync.dma_start(out=wht, in_=whc[n])

                if flags["use_bh"]:
                    bh_bc = bh_pool.tile([P, NV], f32)
                    src = bass.AP(tensor=bhv.tensor,
                                  offset=bhv.offset + NV * n * 4,
                                  ap=[[0, P], [4, NV]])
                    nc.sync.dma_start(out=bh_bc, in_=src)
                for m in range(4):
                    ps = psum_mm.tile([P, NV], f32, tag="ps")
                    for kk in range(KD // 2):
                        nc.tensor.matmul(
                            ps,
                            lhsT=hc[:, 0, 2 * kk:2 * kk + 2, P * m:P * (m + 1)],
                            rhs=wht[:, 1, 2 * kk:2 * kk + 2, :],
                            start=(kk == 0), stop=False, perf_mode=DRM)
                    for k in range(KD):
                        nc.tensor.matmul(
                            ps,
                            lhsT=hc[:, :, k, P * m:P * (m + 1)],
                            rhs=wht[:, :, k, :],
                            start=False, stop=(k == KD - 1), perf_mode=DRM)
                    ot = out_pool.tile([P, NV], bf16)
                    if flags["use_bh"]:
                        sc = out_pool.tile([P, NV], f32, tag="osc")
                        nc.scalar.activation(sc, ps,
                                             mybir.ActivationFunctionType.Identity,
                                             bias=zero_b, scale=inv_sw)
                        nc.vector.tensor_add(ot, sc, bh_bc)
                    else:
                        nc.scalar.activation(ot, ps,
                                             mybir.ActivationFunctionType.Identity,
                                             bias=zero_b, scale=inv_sw)
                    nc.sync.dma_start(out=out[P * m:P * (m + 1), NV * n:NV * (n + 1)],
                                      in_=ot)

    nc.compile()
    return nc


def _get_program(flags):
    key = tuple(sorted(flags.items()))
    if key not in _CACHE:
        _CACHE[key] = _build_program(flags)
    return _CACHE[key]


def _f8pair(w):
    """w (f32) -> (d8, w8) fp8 e4m3 blocks of SW*w: SW*w ~ w8 + d8."""
    ws = np.clip(w * SW, -240.0, 240.0)
    w8 = ws.astype(E4NP)
    d8 = (ws - w8.astype(np.float32)).astype(E4NP)
    return d8, w8


def kernel(x, Wq, bq, Wk, bk, Wv, bv, Wo, bo, decay_param, g_fast, b_fast,
           W1, b1, W2, b2, g_cms, beta_cms, Wh, bh):
    x = np.asarray(x, np.float32)
    Wq, Wk, Wv, Wo = (np.asarray(a, np.float32) for a in (Wq, Wk, Wv, Wo))
    bq, bk, bv, bo = (np.asarray(a, np.float32) for a in (bq, bk, bv, bo))
    g_fast, b_fast = np.asarray(g_fast, np.float32), np.asarray(b_fast, np.float32)
    W1, W2 = np.asarray(W1, np.float32), np.asarray(W2, np.float32)
    b1, b2 = np.asarray(b1, np.float32), np.asarray(b2, np.float32)
    g_cms, beta_cms = np.asarray(g_cms, np.float32), np.asarray(beta_cms, np.float32)
    Wh, bh = np.asarray(Wh, np.float32), np.asarray(bh, np.float32)
    decay = float(1.0 / (1.0 + np.exp(-np.float64(np.asarray(decay_param)))))
    if decay ** HALO > 1e-12:
        raise NotImplementedError(
            f"decay={decay} too close to 1 for banded attention (halo={HALO})")

    flags = {
        "use_bq": bool(np.any(bq)), "use_bk": bool(np.any(bk)),
        "use_bv": bool(np.any(bv)), "use_bo": bool(np.any(bo)),
        "use_b1": bool(np.any(b1)), "use_b2": bool(np.any(b2)),
        "use_bh": bool(np.any(bh)),
        "use_gbf": bool(np.any(g_fast != 1.0) or np.any(b_fast)),
        "use_gbc": bool(np.any(g_cms != 1.0) or np.any(beta_cms)),
    }
    nc = _get_program(flags)

    # host-side weight layout prep (shared by all cores)
    wq_h = np.ascontiguousarray(Wq.reshape(D, KD, P).transpose(1, 0, 2))
    wk_h = np.ascontiguousarray(Wk.reshape(D, KD, P).transpose(1, 0, 2))
    wo_h = np.ascontiguousarray(Wo.reshape(D, KD, P).transpose(1, 0, 2))

    # fp8 pairs, SBUF-layout (partition-major): [l, m, p, b, k, c]
    d1, w1_8 = _f8pair(W1)                      # [L, D, ED]
    a = w1_8.reshape(L, KD, P, KU, P).transpose(0, 3, 2, 1, 4)
    d = d1.reshape(L, KD, P, KU, P).transpose(0, 3, 2, 1, 4)
    w1c_h = np.stack([d, a], axis=3)                            # [L,KU,P,2,KD,P]
    w1c_h = np.ascontiguousarray(
        w1c_h.reshape(L, KU // 4, 4, P, 2, KD, P).transpose(0, 1, 3, 2, 4, 5, 6))
    d2, w2_8 = _f8pair(W2)                      # [L, ED, D]
    a = w2_8.reshape(L, KU, P, KD, P).transpose(0, 3, 2, 1, 4)
    d = d2.reshape(L, KU, P, KD, P).transpose(0, 3, 2, 1, 4)
    w2c_h = np.ascontiguousarray(np.stack([d, a], axis=3))      # [L,KD,P,2,KU,P]
    dh_, wh_8 = _f8pair(Wh)                     # [D, V]
    a = wh_8.reshape(KD, P, NVC, NV).transpose(2, 1, 0, 3)      # [n, p, k, nv]
    d = dh_.reshape(KD, P, NVC, NV).transpose(2, 1, 0, 3)
    whc_h = np.ascontiguousarray(np.stack([d, a], axis=2))      # [NVC,P,2,KD,NV]

    smalls = np.zeros((NS, D), np.float32)
    smalls[I_BQ], smalls[I_BK], smalls[I_BV], smalls[I_BO] = bq, bk, bv, bo
    smalls[I_GF], smalls[I_BF] = g_fast, b_fast
    for l in range(L):
        smalls[I_B2(l)], smalls[I_GC(l)], smalls[I_BE(l)] = b2[l], g_cms[l], beta_cms[l]
    smalls = np.ascontiguousarray(
        smalls.reshape(NS, KD, P).transpose(2, 0, 1))        # [P, NS, KD]
    b1_r = np.ascontiguousarray(
        b1.reshape(L, KU, P).transpose(2, 0, 1))             # [P, L, KU]

    shared = {"wq": wq_h, "wk": wk_h, "wo": wo_h, "wv": Wv,
              "w1c": w1c_h, "w2c": w2c_h, "whc": whc_h,
              "smalls": smalls, "b1v": b1_r,
              "onesc": np.ones((P, 1), np.float32),
              "onesr": np.ones((1, P), np.float32)}
    if flags["use_bh"]:
        shared["bhv"] = bh
    if flags["use_bv"]:
        shared["bvv"] = bv

    in_maps = []
    i_loc = np.arange(CH)[:, None]
    j_loc = np.arange(WIN)[None, :]
    for c in range(NCORES):
        bidx, start = divmod(c * CH, S)
        w0 = start - HALO
        xwin = np.zeros((WIN, D), np.float32)
        lo = max(w0, 0)
        xwin[lo - w0:, :] = x[bidx, lo:start + CH]
        xw_h = np.ascontiguousarray(xwin.T)  # [D, WIN]
        expo = (HALO + i_loc) - j_loc - 1
        valid = (expo >= 0) & (w0 + j_loc >= 0)
        maskq = np.where(valid,
                         np.power(np.float64(decay), np.maximum(expo, 0)),
                         0.0).astype(np.float32)          # [CH, WIN]
        mask_h = np.ascontiguousarray(maskq.T)            # [WIN, CH]
        in_maps.append({**shared, "xw": xw_h, "maskT": mask_h})

    res = run_bass_kernel_spmd(nc, in_maps, list(range(NCORES)), trace=TRACE)
    outs = [res.results[c]["out"].astype(np.float32) for c in range(NCORES)]
    full = np.concatenate(outs, axis=0).reshape(B, S, V)
    kernel.last_result = res
    return full
